# revision 1
# baseline (speedup 1.0000x reference)
"""LSTM warmup + autoregressive decode kernel for Trainium2 (Bass/Tile).

Reference computation (per batch row):
  h,c = 0
  for t in range(T):  h,c = LSTMstep(x_t)        # warmup over input seq
  pred0 = h @ Wd + bd
  for d in range(out_steps-1): h,c = LSTMstep(pred_d); pred_{d+1} = h@Wd+bd
  out[b, s, f] = pred_s

Strategy: data-parallel over 8 NeuronCores (B=4096 -> 512/core); the
sequential time loop stays local per shard.  On-chip everything is kept in a
*transposed* layout (partitions = unit/feature index, free dim = batch):
z^T[1024, B] per step via fp32r matmuls (weights stationary, x^T/h^T moving),
gates as [128, 2*B] tiles, so h^T feeds the next step's matmuls directly and
the recurrence needs no transposes.  PE transposes (via identity) only stage
x^T from the input layout and emit the output layout.

The autoregressive decode is algebraically fused: since
  z_{t+1} = pred_t @ W + h_t @ U + b   and   pred_t = h_t @ Wd + bd,
we precompute Ud = U + Wd@W and bias b + bd@W on the host, making each decode
step a single K=256 recurrence with no pred -> x round trip on the critical
path (pred is still computed, but only as output staging).
"""

import sys

for _p in ("/opt/trn_rl_repo", "/root/.axon_site/_ro/trn_rl_repo"):
    if _p not in sys.path:
        sys.path.insert(0, _p)

import numpy as np

import concourse.bacc as bacc
import concourse.mybir as mybir
import concourse.tile as tile
from concourse import bass_utils

F32 = mybir.dt.float32
F32R = mybir.dt.float32r
AF = mybir.ActivationFunctionType

N_CORES = 8
F = 64          # input/output feature dim
U = 256         # lstm units
U4 = 4 * U      # gate rows
# gate order in the 1024-row z layout (keras order i,f,g,o)
G_I, G_F, G_G, G_O = 0, 1, 2, 3


def build_program(B, T, out_steps, use_f32r=True):
    """Build the single-core SPMD program for a batch shard of size B."""
    assert B % 128 == 0
    NB = B // 128
    assert T % 2 == 0
    n_in_pairs = T // 2

    nc = bacc.Bacc("TRN2", target_bir_lowering=False, debug=False, num_devices=1)

    WDT = F32R if use_f32r else F32
    xin = nc.dram_tensor("xin", [B, T, F], F32, kind="ExternalInput").ap()
    w2d = nc.dram_tensor("w2", [128, U4], WDT, kind="ExternalInput").ap()
    u2d = nc.dram_tensor("u2", [128, 2 * U4], WDT, kind="ExternalInput").ap()
    ud2d = nc.dram_tensor("ud2", [128, 2 * U4], WDT, kind="ExternalInput").ap()
    wdd_d = nc.dram_tensor("wdd", [128, 2 * F], WDT, kind="ExternalInput").ap()
    ident_d = nc.dram_tensor("ident", [128, 128], F32, kind="ExternalInput").ap()
    bias8_d = nc.dram_tensor("bias8", [128, 8], F32, kind="ExternalInput").ap()
    bias8d_d = nc.dram_tensor("bias8d", [128, 8], F32, kind="ExternalInput").ap()
    bdup_d = nc.dram_tensor("bdup", [128, 1], F32, kind="ExternalInput").ap()
    yout = nc.dram_tensor("yout", [B, out_steps, F], F32, kind="ExternalOutput").ap()

    xin_f = xin.rearrange("b t f -> b (t f)")
    yout_f = yout.rearrange("b s f -> b (s f)")

    def mmt(ap):
        return ap.bitcast(F32R) if use_f32r else ap

    rnd = mmt  # producers feeding fp32r matmuls must declare fp32r outputs

    with tile.TileContext(nc) as tc:
        import contextlib

        with contextlib.ExitStack() as ctx:
            wpool = ctx.enter_context(tc.tile_pool(name="wpool", bufs=1))
            dpool = ctx.enter_context(tc.tile_pool(name="dpool", bufs=8))
            xpool = ctx.enter_context(tc.tile_pool(name="xpool", bufs=6))
            gpool = ctx.enter_context(tc.tile_pool(name="gpool", bufs=2))
            opool = ctx.enter_context(tc.tile_pool(name="opool", bufs=3))
            prpool = ctx.enter_context(tc.tile_pool(name="prpool", bufs=4))
            zpool = ctx.enter_context(tc.tile_pool(name="zpool", bufs=6, space="PSUM"))
            upool = ctx.enter_context(tc.tile_pool(name="upool", bufs=2, space="PSUM"))

            # ---- constants / weights ----
            w2 = wpool.tile([128, U4], WDT)        # W duplicated rows 0:64 / 64:128
            nc.sync.dma_start(w2[:], w2d[:])
            u2 = wpool.tile([128, 2 * U4], WDT)    # warmup U, k-chunks side by side
            nc.sync.dma_start(u2[:], u2d[:])
            ud2 = wpool.tile([128, 2 * U4], WDT)   # decode U + Wd@W
            nc.sync.dma_start(ud2[:], ud2d[:])
            wdd = wpool.tile([128, 2 * F], WDT)    # Wd k-chunks side by side
            nc.sync.dma_start(wdd[:], wdd_d[:])
            ident = wpool.tile([128, 128], F32)
            nc.sync.dma_start(ident[:], ident_d[:])
            bias8 = wpool.tile([128, 8], F32)
            nc.sync.dma_start(bias8[:], bias8_d[:])
            bias8d = wpool.tile([128, 8], F32)
            nc.sync.dma_start(bias8d[:], bias8d_d[:])
            bdup = wpool.tile([128, 1], F32)
            nc.sync.dma_start(bdup[:], bdup_d[:])

            xpairs = {}   # pair idx -> SBUF [128, B] x^T for steps 2p, 2p+1
            preds = {}    # decode pred idx -> SBUF [64, B] pred^T

            def emit_in_pair(p):
                xp = upool.tile([128, B], F32, tag="util", name=f"xtp{p}")
                for bc in range(NB):
                    dt_in = dpool.tile([128, 128], F32, tag="din", name=f"din{p}_{bc}")
                    nc.sync.dma_start(
                        dt_in[:],
                        xin_f[128 * bc : 128 * (bc + 1), 128 * p : 128 * (p + 1)],
                    )
                    nc.tensor.transpose(
                        xp[:, 128 * bc : 128 * (bc + 1)], dt_in[:], ident[:]
                    )
                xs = xpool.tile([128, B], F32, tag="xpair", name=f"xpair{p}")
                nc.vector.tensor_copy(rnd(xs[:]), xp[:])
                xpairs[p] = xs

            GATES = ((G_I, AF.Sigmoid, "gi"), (G_G, AF.Tanh, "gg"),
                     (G_F, AF.Sigmoid, "gf"), (G_O, AF.Sigmoid, "go"))

            def lstm_step(t, h_prev, c_prev, x_src=None, rb=0):
                """One LSTM step.  decode (x_src None): fused Ud recurrence.
                Returns (h, c) tiles [128, 2*B] in (uchunk, batch) layout."""
                uw = u2 if x_src is not None else ud2
                bias = bias8 if x_src is not None else bias8d
                zt = {}
                for ch in (0, 1):
                    for q, _, _ in GATES:
                        zq = zpool.tile([128, B], F32, tag="z", name=f"z{t}_{q}_{ch}")
                        mcol = 256 * q + 128 * ch
                        first = True
                        if x_src is not None:
                            nc.tensor.matmul(
                                zq[:],
                                mmt(w2[rb : rb + 64, mcol : mcol + 128]),
                                mmt(x_src[rb : rb + 64, :]),
                                start=True,
                                stop=(h_prev is None),
                            )
                            first = False
                        if h_prev is not None:
                            nc.tensor.matmul(
                                zq[:],
                                mmt(uw[:, mcol : mcol + 128]),
                                mmt(h_prev[:, 0:B]),
                                start=first,
                                stop=False,
                            )
                            nc.tensor.matmul(
                                zq[:],
                                mmt(uw[:, U4 + mcol : U4 + mcol + 128]),
                                mmt(h_prev[:, B : 2 * B]),
                                start=False,
                                stop=True,
                            )
                        zt[(q, ch)] = zq

                g = {}
                for q, _, tg in GATES:
                    g[q] = gpool.tile([128, 2 * B], F32, tag=tg, name=f"g{t}_{q}")
                c_t = gpool.tile([128, 2 * B], F32, tag="c", name=f"c{t}")
                tc_t = gpool.tile([128, 2 * B], F32, tag="tc", name=f"tc{t}")
                h_t = gpool.tile([128, 2 * B], F32, tag="h", name=f"h{t}")
                m2 = gpool.tile([128, 2 * B], F32, tag="m2", name=f"m2_{t}")
                if c_prev is not None:
                    fc = gpool.tile([128, 2 * B], F32, tag="fc", name=f"fc{t}")

                for ch in (0, 1):
                    s = slice(B * ch, B * (ch + 1))
                    for q, func, _ in GATES:
                        bcol = 2 * q + ch
                        nc.scalar.activation(
                            g[q][:, s], zt[(q, ch)][:],
                            func, bias=bias[:, bcol : bcol + 1],
                        )
                        if q == G_G:
                            nc.vector.tensor_mul(m2[:, s], g[G_I][:, s], g[G_G][:, s])
                        elif q == G_F and c_prev is not None:
                            nc.vector.tensor_mul(fc[:, s], g[G_F][:, s], c_prev[:, s])
                            nc.vector.tensor_add(c_t[:, s], fc[:, s], m2[:, s])
                    if c_prev is None:
                        nc.vector.tensor_copy(c_t[:, s], m2[:, s])
                    nc.scalar.activation(tc_t[:, s], c_t[:, s], AF.Tanh)
                    nc.vector.tensor_mul(rnd(h_t[:, s]), g[G_O][:, s], tc_t[:, s])
                return h_t, c_t

            def emit_pred(d, h_t):
                """pred_d^T = Wd^T h + bd -> [64, B] SBUF tile."""
                pp = upool.tile([64, B], F32, tag="util", name=f"predp{d}")
                nc.tensor.matmul(
                    pp[:], mmt(wdd[:, 0:F]), mmt(h_t[:, 0:B]), start=True, stop=False
                )
                nc.tensor.matmul(
                    pp[:], mmt(wdd[:, F : 2 * F]), mmt(h_t[:, B : 2 * B]),
                    start=False, stop=True,
                )
                ps = prpool.tile([64, B], F32, tag="prp", name=f"prsb{d}")
                nc.scalar.activation(ps[:], pp[:], AF.Identity, bias=bdup[0:64, 0:1])
                preds[d] = ps

            def emit_out_step(d):
                """Transpose pred_d to [batch, feat] layout and DMA out."""
                ps = preds.pop(d)
                tp = upool.tile([128, NB * F], F32, tag="util", name=f"otp{d}")
                for bc in range(NB):
                    nc.tensor.matmul(
                        tp[:, F * bc : F * (bc + 1)],
                        ps[:, 128 * bc : 128 * (bc + 1)],
                        ident[0:64, 0:F],
                        is_transpose=True,
                    )
                osb = opool.tile([128, NB * F], F32, tag="ot", name=f"osb{d}")
                nc.vector.tensor_copy(osb[:], tp[:])
                for bc in range(NB):
                    nc.sync.dma_start(
                        yout_f[128 * bc : 128 * (bc + 1), F * d : F * (d + 1)],
                        osb[:, F * bc : F * bc + F],
                    )

            # ---- warmup over the input sequence ----
            emit_in_pair(0)
            if n_in_pairs > 1:
                emit_in_pair(1)
            h_t = c_t = None
            for t in range(T):
                p, rb = t // 2, 64 * (t % 2)
                if t % 2 == 0 and p + 2 < n_in_pairs:
                    emit_in_pair(p + 2)
                h_t, c_t = lstm_step(t, h_t, c_t, x_src=xpairs[p], rb=rb)
                if t % 2 == 1:
                    del xpairs[p]

            # ---- autoregressive decode (fused recurrence) ----
            # pred_k/output emission lags one step so the recurrence matmuls
            # keep scheduling priority.
            hs = {0: h_t}
            for k in range(1, out_steps):
                h_t, c_t = lstm_step(T + k, h_t, c_t)
                hs[k] = h_t
                emit_pred(k - 1, hs.pop(k - 1))
                if k >= 2:
                    emit_out_step(k - 2)
            emit_pred(out_steps - 1, hs.pop(out_steps - 1))
            if out_steps >= 2:
                emit_out_step(out_steps - 2)
            emit_out_step(out_steps - 1)

    nc.compile()
    return nc


_CACHE = {}


def _get_program(key):
    if key not in _CACHE:
        _CACHE[key] = build_program(*key)
    return _CACHE[key]


def _host_prep(W, Uk, b, Wd, bd):
    W64 = W.astype(np.float64)
    Ud = (Uk.astype(np.float64) + Wd.astype(np.float64) @ W64).astype(np.float32)
    bdec = (b.astype(np.float64) + bd.astype(np.float64) @ W64).astype(np.float32)
    w2 = np.concatenate([W, W], axis=0).astype(np.float32)            # [128, 1024]
    u2 = np.concatenate([Uk[0:128], Uk[128:256]], axis=1).astype(np.float32)
    ud2 = np.concatenate([Ud[0:128], Ud[128:256]], axis=1).astype(np.float32)
    wdd = np.concatenate([Wd[0:128], Wd[128:256]], axis=1).astype(np.float32)
    ident = np.eye(128, dtype=np.float32)
    bias8 = np.ascontiguousarray(b.reshape(8, 128).T.astype(np.float32))
    bias8d = np.ascontiguousarray(bdec.reshape(8, 128).T.astype(np.float32))
    bdup = np.concatenate([bd, bd]).reshape(128, 1).astype(np.float32)
    return {
        "w2": w2, "u2": u2, "ud2": ud2, "wdd": wdd, "ident": ident,
        "bias8": bias8, "bias8d": bias8d, "bdup": bdup,
    }


def kernel(inputs, W, U, b, Wd, bd, out_steps):
    inputs = np.asarray(inputs, dtype=np.float32)
    W = np.asarray(W, dtype=np.float32)
    U_ = np.asarray(U, dtype=np.float32)
    b_ = np.asarray(b, dtype=np.float32)
    Wd = np.asarray(Wd, dtype=np.float32)
    bd = np.asarray(bd, dtype=np.float32)
    out_steps = int(out_steps)

    B_full, T, _ = inputs.shape
    assert B_full % N_CORES == 0
    Bc = B_full // N_CORES

    nc = _get_program((Bc, T, out_steps, True))
    shared = _host_prep(W, U_, b_, Wd, bd)
    in_maps = [
        {"xin": np.ascontiguousarray(inputs[i * Bc : (i + 1) * Bc]), **shared}
        for i in range(N_CORES)
    ]
    res = bass_utils.run_bass_kernel_spmd(nc, in_maps, core_ids=list(range(N_CORES)))
    out = np.concatenate([res.results[i]["yout"] for i in range(N_CORES)], axis=0)
    return out



# revision 15
# speedup vs baseline: 96.0867x; 96.0867x over previous
"""LSTM warmup + autoregressive decode kernel for Trainium2 (Bass/Tile).

Reference computation (per batch row):
  h,c = 0
  for t in range(T):  h,c = LSTMstep(x_t)        # warmup over input seq
  pred0 = h @ Wd + bd
  for d in range(out_steps-1): h,c = LSTMstep(pred_d); pred_{d+1} = h@Wd+bd
  out[b, s, f] = pred_s

Data-parallel over 8 NeuronCores (B=4096 -> 512/core).  On-chip layout is
transposed (partitions = unit/feature index, free dim = batch) so the
recurrence h^T feeds the next step's matmuls with no per-step transposes.

Key optimizations over the v1 baseline (the recurrence is latency-bound on
the single Activation engine -- 95 sequential steps, each needing 4 gate
activations + tanh(c)):
  * per-gate z tiles are [128, 2B] (both unit-halves), so each gate needs ONE
    activation instruction per step (6 total incl. split tanh(c)) instead of 10.
  * gate biases are folded into the matmuls (ones-row appended to x^T /
    K=1 bias matmul in decode), which is what makes the merged per-gate
    activation legal (bias would otherwise differ across the two unit-halves
    on the same partition).
  * all matmuls run in bf16 (1 cycle/row on the PE at any width); gate values
    and h are bf16 (2x DVE throughput on i*g and o*tanh(c)); c stays fp32.
  * the dense/output path (pred = h@Wd+bd, transpose, DMA) is done entirely
    with PE matmuls + Pool-engine copies + direct PSUM->HBM DMA: zero
    Activation or Vector engine time.
  * autoregressive decode is algebraically fused: Ud = U + Wd@W, so each
    decode step is a single K=256 recurrence with no pred -> x round trip.
  * PE instruction order pipelines next-step x-projections and input
    transposes into the recurrence tail so the PE never blocks the chain.
"""

import sys

for _p in ("/opt/trn_rl_repo", "/root/.axon_site/_ro/trn_rl_repo"):
    if _p not in sys.path:
        sys.path.insert(0, _p)

import numpy as np

import concourse.bacc as bacc
import concourse.mybir as mybir
import concourse.tile as tile
from concourse import bass_utils

F32 = mybir.dt.float32
BF16 = mybir.dt.bfloat16
AF = mybir.ActivationFunctionType

N_CORES = 8
F = 64          # input/output feature dim
U = 256         # lstm units
U4 = 4 * U      # gate rows
# gate order in the 1024-col z layout (keras order i,f,g,o)
G_I, G_F, G_G, G_O = 0, 1, 2, 3


def build_program(B, T, out_steps):
    """Single-core SPMD program for a batch shard of size B (=512)."""
    assert B % 128 == 0
    NB = B // 128
    B2 = 2 * B
    WIN = 10            # x^T staging lookahead (steps)

    nc = bacc.Bacc("TRN2", target_bir_lowering=False, debug=False, num_devices=1)

    xin = nc.dram_tensor("xin", [B, T, F], F32, kind="ExternalInput").ap()
    wext_d = nc.dram_tensor("wext", [F + 1, U4], BF16, kind="ExternalInput").ap()
    u2_d = nc.dram_tensor("u2", [128, 2 * U4], BF16, kind="ExternalInput").ap()
    ud2_d = nc.dram_tensor("ud2", [128, 2 * U4], BF16, kind="ExternalInput").ap()
    bdec_d = nc.dram_tensor("bdec", [1, U4], BF16, kind="ExternalInput").ap()
    wdd_d = nc.dram_tensor("wdd", [128, 2 * F], BF16, kind="ExternalInput").ap()
    bdrow_d = nc.dram_tensor("bdrow", [1, F], BF16, kind="ExternalInput").ap()
    ident_d = nc.dram_tensor("ident", [128, 128], F32, kind="ExternalInput").ap()
    yout = nc.dram_tensor("yout", [B, out_steps, F], F32, kind="ExternalOutput").ap()

    xin_f = xin.rearrange("b t f -> b (t f)")
    yout_f = yout.rearrange("b s f -> b (s f)")

    n_steps = T + out_steps - 1          # total lstm steps (h(T-1+j) -> pred_j)

    with tile.TileContext(nc) as tc:
        import contextlib

        with contextlib.ExitStack() as ctx:
            wpool = ctx.enter_context(tc.tile_pool(name="wpool", bufs=1))
            dpool = ctx.enter_context(tc.tile_pool(name="dpool", bufs=8))
            xspool = ctx.enter_context(tc.tile_pool(name="xspool", bufs=WIN + 2))
            gpool = ctx.enter_context(tc.tile_pool(name="gpool", bufs=2))
            pspool = ctx.enter_context(tc.tile_pool(name="pspool", bufs=2))
            zpool = ctx.enter_context(tc.tile_pool(name="zpool", bufs=3, space="PSUM"))
            upool = ctx.enter_context(tc.tile_pool(name="upool", bufs=2, space="PSUM"))

            # ---- constants / weights ----
            wext = wpool.tile([F + 1, U4], BF16)
            nc.sync.dma_start(wext[:], wext_d[:])
            u2 = wpool.tile([128, 2 * U4], BF16)
            nc.sync.dma_start(u2[:], u2_d[:])
            ud2 = wpool.tile([128, 2 * U4], BF16)
            nc.sync.dma_start(ud2[:], ud2_d[:])
            bdec = wpool.tile([1, U4], BF16)
            nc.sync.dma_start(bdec[:], bdec_d[:])
            wdd = wpool.tile([128, 2 * F], BF16)
            nc.sync.dma_start(wdd[:], wdd_d[:])
            bdrow = wpool.tile([1, F], BF16)
            nc.sync.dma_start(bdrow[:], bdrow_d[:])
            ident = wpool.tile([128, 128], F32)
            nc.sync.dma_start(ident[:], ident_d[:])
            ones = wpool.tile([1, B], BF16)
            nc.gpsimd.memset(ones[:], 1.0)

            dts = {}      # pair idx -> list of 4 dt tiles [128,128]
            xss = {}      # step -> xs tile [65, B] bf16 (x^T + ones row)

            def load_pair(p):
                tiles = []
                for bc in range(NB):
                    dt = dpool.tile([128, 128], F32, tag="dt", name=f"dt{p}_{bc}")
                    nc.sync.dma_start(
                        dt[:], xin_f[128 * bc : 128 * (bc + 1), 128 * p : 128 * (p + 1)]
                    )
                    tiles.append(dt)
                dts[p] = tiles

            def stage_step(s):
                """Transpose x_s into xs[s] = [F+1, B] bf16 (ones row at F)."""
                p, half = s // 2, s % 2
                st = upool.tile([F, B], F32, tag="u", name=f"st{s}")
                for bc in range(NB):
                    nc.tensor.transpose(
                        st[:, 128 * bc : 128 * (bc + 1)],
                        dts[p][bc][:, F * half : F * (half + 1)],
                        ident[:],
                    )
                xs = xspool.tile([F + 1, B], BF16, tag="xs", name=f"xs{s}")
                nc.gpsimd.memset(xs[F : F + 1, :], 1.0)
                nc.vector.tensor_copy(xs[0:F, :], st[:])
                xss[s] = xs
                if half == 1:
                    del dts[p]

            # gate order on PE / Act: f, i, g, o
            GATES = (G_F, G_I, G_G, G_O)
            zt = {}       # (step, gate) -> z tile [128, 2B] psum

            def mcol(q, ch):
                return 256 * q + 128 * ch

            def emit_zstart(t, q):
                """Allocate z tile for (t, q) and emit its bias/x matmul(s)."""
                zq = zpool.tile([128, B2], F32, tag="z", name=f"z{t}_{q}")
                zt[(t, q)] = zq
                last = t == 0       # step 0 has no recurrent term
                if t < T:
                    for ch in (0, 1):
                        nc.tensor.matmul(
                            zq[:, B * ch : B * (ch + 1)],
                            wext[:, mcol(q, ch) : mcol(q, ch) + 128],
                            xss[t][:],
                            start=True,
                            stop=last,
                        )
                else:
                    for ch in (0, 1):
                        nc.tensor.matmul(
                            zq[:, B * ch : B * (ch + 1)],
                            bdec[:, mcol(q, ch) : mcol(q, ch) + 128],
                            ones[:],
                            start=True,
                            stop=False,
                        )
                return zq

            def emit_kmms(t, q, h_prev):
                """Recurrent matmuls for gate q of step t.  For the f gate the
                ch0 output column is finished first so sig(f0) can start before
                h1 of the previous step is even ready."""
                uw = u2 if t < T else ud2
                zq = zt[(t, q)]
                for k, ch in ((0, 0), (0, 1), (1, 0), (1, 1)):
                    nc.tensor.matmul(
                        zq[:, B * ch : B * (ch + 1)],
                        uw[:, U4 * k + mcol(q, ch) : U4 * k + mcol(q, ch) + 128],
                        h_prev[:, B * k : B * (k + 1)],
                        start=False,
                        stop=(k == 1),
                    )

            def emit_act_dve(t, c_prev):
                """Gate activations + cell update for step t. Returns (h, c)."""
                sf = gpool.tile([128, B2], BF16, tag="sf", name=f"sf{t}")
                si = gpool.tile([128, B2], BF16, tag="si", name=f"si{t}")
                tg = gpool.tile([128, B2], BF16, tag="tg", name=f"tg{t}")
                so = gpool.tile([128, B2], BF16, tag="so", name=f"so{t}")
                th = gpool.tile([128, B2], BF16, tag="th", name=f"th{t}")
                t1 = gpool.tile([128, B2], BF16, tag="t1", name=f"t1_{t}")
                fc = gpool.tile([128, B2], F32, tag="fc", name=f"fc{t}")
                c_t = gpool.tile([128, B2], F32, tag="c", name=f"c{t}")
                h_t = gpool.tile([128, B2], BF16, tag="h", name=f"h{t}")

                # Act stream (chain-gating sig(f)/tanh(g) are ch-split so the
                # cell update can start as early as possible):
                s0 = slice(0, B)
                s1 = slice(B, B2)
                nc.scalar.activation(sf[:], zt.pop((t, G_F))[:], AF.Sigmoid)
                nc.scalar.activation(si[:], zt.pop((t, G_I))[:], AF.Sigmoid)
                nc.scalar.activation(tg[:], zt.pop((t, G_G))[:], AF.Tanh)
                nc.scalar.activation(so[:], zt.pop((t, G_O))[:], AF.Sigmoid)
                if c_prev is not None:
                    nc.vector.tensor_mul(fc[:, s0], sf[:, s0], c_prev[:, s0])
                    nc.vector.tensor_mul(fc[:, s1], sf[:, s1], c_prev[:, s1])
                    nc.vector.tensor_mul(t1[:, s0], si[:, s0], tg[:, s0])
                    nc.vector.tensor_add(c_t[:, s0], fc[:, s0], t1[:, s0])
                    nc.vector.tensor_mul(t1[:, s1], si[:, s1], tg[:, s1])
                    nc.vector.tensor_add(c_t[:, s1], fc[:, s1], t1[:, s1])
                else:
                    nc.vector.tensor_mul(c_t[:, s0], si[:, s0], tg[:, s0])
                    nc.vector.tensor_mul(c_t[:, s1], si[:, s1], tg[:, s1])
                nc.scalar.activation(th[:, s0], c_t[:, s0], AF.Tanh)
                nc.vector.tensor_mul(h_t[:, s0], so[:, s0], th[:, s0])
                nc.scalar.activation(th[:, s1], c_t[:, s1], AF.Tanh)
                nc.vector.tensor_mul(h_t[:, s1], so[:, s1], th[:, s1])
                return h_t, c_t

            preds = {}    # j -> ps tile [F, B] f32 (pred_j^T, bias included)

            def emit_pred(j, h_t):
                """pred_j^T = Wd^T h + bd -> SBUF [F, B] via PE + Pool only."""
                pp = upool.tile([F, B], F32, tag="u", name=f"pp{j}")
                nc.tensor.matmul(pp[:], bdrow[:], ones[:], start=True, stop=False)
                nc.tensor.matmul(pp[:], wdd[:, 0:F], h_t[:, 0:B], start=False, stop=False)
                nc.tensor.matmul(
                    pp[:], wdd[:, F : 2 * F], h_t[:, B:B2], start=False, stop=True
                )
                ps = pspool.tile([F, B], F32, tag="ps", name=f"ps{j}")
                nc.vector.tensor_copy(ps[:], pp[:])
                preds[j] = ps

            def emit_out(j):
                """Transpose pred_j to [batch, feat] and DMA straight from PSUM."""
                ps = preds.pop(j)
                ot = upool.tile([128, NB * F], F32, tag="u", name=f"ot{j}")
                for bc in range(NB):
                    nc.tensor.transpose(
                        ot[:, F * bc : F * (bc + 1)],
                        ps[:, 128 * bc : 128 * (bc + 1)],
                        ident[0:F, 0:F],
                    )
                osb = pspool.tile([128, NB * F], F32, tag="ob", name=f"ob{j}")
                nc.vector.tensor_copy(osb[:], ot[:])
                for bc in range(NB):
                    nc.sync.dma_start(
                        yout_f[128 * bc : 128 * (bc + 1), F * j : F * (j + 1)],
                        osb[:, F * bc : F * bc + F],
                    )

            # ================= prologue =================
            for p in range((WIN + 1) // 2 + 1):
                load_pair(p)
            for s in range(WIN):
                stage_step(s)
            for q in GATES:
                emit_zstart(0, q)

            # ================= main loop =================
            h_t = c_t = None
            for t in range(n_steps):
                h_prev, c_prev = h_t, c_t
                if t > 0:
                    # k-chunk matmuls for f, i, g; o's z-start (slot frees after
                    # sig(f) of this step) then o's k-matmuls.
                    emit_kmms(t, G_F, h_prev)
                    emit_kmms(t, G_I, h_prev)
                    emit_kmms(t, G_G, h_prev)
                    emit_zstart(t, G_O)
                    emit_kmms(t, G_O, h_prev)

                # PE filler work for the recurrence tail: input staging,
                # decode output path, next step's bias/x matmuls.
                s = t + WIN
                if s < T:
                    if s % 2 == 0 and s // 2 + 1 < (T + 1) // 2:
                        load_pair(s // 2 + 1)
                    stage_step(s)
                if t > T:
                    emit_pred(t - T, h_prev)        # pred_{t-T} from h(t-1)
                elif t == T:
                    emit_pred(0, h_prev)            # pred_0 from warmup-final h
                if t > T and t - T >= 2:
                    emit_out(t - T - 2)
                if t + 1 < n_steps:
                    emit_zstart(t + 1, G_F)
                    emit_zstart(t + 1, G_I)
                    emit_zstart(t + 1, G_G)

                h_t, c_t = emit_act_dve(t, c_prev)

            # ================= epilogue =================
            emit_pred(out_steps - 1, h_t)
            for j in (out_steps - 3, out_steps - 2, out_steps - 1):
                if j >= 0 and j in preds:
                    emit_out(j)

    nc.compile()
    return nc


_CACHE = {}


def _get_program(key):
    if key not in _CACHE:
        _CACHE[key] = build_program(*key)
    return _CACHE[key]


def _host_prep(W, Uk, b, Wd, bd):
    bf = mybir.dt.np(BF16)
    W64 = W.astype(np.float64)
    Ud = (Uk.astype(np.float64) + Wd.astype(np.float64) @ W64).astype(np.float32)
    bdec = (b.astype(np.float64) + bd.astype(np.float64) @ W64).astype(np.float32)
    wext = np.concatenate([W, b.reshape(1, -1)], axis=0)          # [65, 1024]
    u2 = np.concatenate([Uk[0:128], Uk[128:256]], axis=1)         # [128, 2048]
    ud2 = np.concatenate([Ud[0:128], Ud[128:256]], axis=1)
    wdd = np.concatenate([Wd[0:128], Wd[128:256]], axis=1)        # [128, 128]
    ident = np.eye(128, dtype=np.float32)
    return {
        "wext": wext.astype(bf),
        "u2": u2.astype(bf),
        "ud2": ud2.astype(bf),
        "bdec": np.ascontiguousarray(bdec.reshape(1, -1)).astype(bf),
        "wdd": wdd.astype(bf),
        "bdrow": np.ascontiguousarray(bd.reshape(1, -1)).astype(bf),
        "ident": ident,
    }


def kernel(inputs, W, U, b, Wd, bd, out_steps):
    inputs = np.asarray(inputs, dtype=np.float32)
    W = np.asarray(W, dtype=np.float32)
    U_ = np.asarray(U, dtype=np.float32)
    b_ = np.asarray(b, dtype=np.float32)
    Wd = np.asarray(Wd, dtype=np.float32)
    bd = np.asarray(bd, dtype=np.float32)
    out_steps = int(out_steps)

    B_full, T, _ = inputs.shape
    assert B_full % N_CORES == 0
    Bc = B_full // N_CORES

    nc = _get_program((Bc, T, out_steps))
    shared = _host_prep(W, U_, b_, Wd, bd)
    in_maps = [
        {"xin": np.ascontiguousarray(inputs[i * Bc : (i + 1) * Bc]), **shared}
        for i in range(N_CORES)
    ]
    res = bass_utils.run_bass_kernel_spmd(nc, in_maps, core_ids=list(range(N_CORES)))
    out = np.concatenate([res.results[i]["yout"] for i in range(N_CORES)], axis=0)
    return out


# revision 39
# speedup vs baseline: 97.5142x; 1.0149x over previous
"""LSTM warmup + autoregressive decode kernel for Trainium2 (Bass/Tile).

Reference computation (per batch row):
  h,c = 0
  for t in range(T):  h,c = LSTMstep(x_t)        # warmup over input seq
  pred0 = h @ Wd + bd
  for d in range(out_steps-1): h,c = LSTMstep(pred_d); pred_{d+1} = h@Wd+bd
  out[b, s, f] = pred_s

Data-parallel over 8 NeuronCores (B=4096 -> 512/core).  On-chip layout is
transposed (partitions = unit/feature index, free dim = batch) so the
recurrence h^T feeds the next step's matmuls with no per-step transposes.

Key optimizations over the v1 baseline (the recurrence is latency-bound on
the single Activation engine -- 95 sequential steps, each needing 4 gate
activations + tanh(c)):
  * per-gate z tiles are [128, 2B] (both unit-halves), so each gate needs ONE
    activation instruction per step (6 total incl. split tanh(c)) instead of 10.
  * gate biases are folded into the matmuls (ones-row appended to x^T /
    K=1 bias matmul in decode), which is what makes the merged per-gate
    activation legal (bias would otherwise differ across the two unit-halves
    on the same partition).
  * all matmuls run in bf16 (1 cycle/row on the PE at any width); gate values
    and h are bf16 (2x DVE throughput on i*g and o*tanh(c)); c stays fp32.
  * the dense/output path (pred = h@Wd+bd, transpose, DMA) is done entirely
    with PE matmuls + Pool-engine copies + direct PSUM->HBM DMA: zero
    Activation or Vector engine time.
  * autoregressive decode is algebraically fused: Ud = U + Wd@W, so each
    decode step is a single K=256 recurrence with no pred -> x round trip.
  * PE instruction order pipelines next-step x-projections and input
    transposes into the recurrence tail so the PE never blocks the chain.
"""

import sys

for _p in ("/opt/trn_rl_repo", "/root/.axon_site/_ro/trn_rl_repo"):
    if _p not in sys.path:
        sys.path.insert(0, _p)

import numpy as np

import concourse.bacc as bacc
import concourse.mybir as mybir
import concourse.tile as tile
from concourse import bass_utils

F32 = mybir.dt.float32
BF16 = mybir.dt.bfloat16
AF = mybir.ActivationFunctionType

N_CORES = 8
F = 64          # input/output feature dim
U = 256         # lstm units
U4 = 4 * U      # gate rows
# gate order in the 1024-col z layout (keras order i,f,g,o)
G_I, G_F, G_G, G_O = 0, 1, 2, 3


def build_program(B, T, out_steps):
    """Single-core SPMD program for a batch shard of size B (=512)."""
    assert B % 128 == 0
    NB = B // 128
    B2 = 2 * B
    WIN = 10            # x^T staging lookahead (steps)

    nc = bacc.Bacc("TRN2", target_bir_lowering=False, debug=False, num_devices=1)

    xin = nc.dram_tensor("xin", [B, T, F], F32, kind="ExternalInput").ap()
    wext_d = nc.dram_tensor("wext", [F + 1, U4], BF16, kind="ExternalInput").ap()
    u2_d = nc.dram_tensor("u2", [128, 2 * U4], BF16, kind="ExternalInput").ap()
    ud2_d = nc.dram_tensor("ud2", [128, 2 * U4], BF16, kind="ExternalInput").ap()
    bdec_d = nc.dram_tensor("bdec", [1, U4], BF16, kind="ExternalInput").ap()
    wdd_d = nc.dram_tensor("wdd", [128, 2 * F], BF16, kind="ExternalInput").ap()
    bdrow_d = nc.dram_tensor("bdrow", [1, F], BF16, kind="ExternalInput").ap()
    ident_d = nc.dram_tensor("ident", [128, 128], F32, kind="ExternalInput").ap()
    yout = nc.dram_tensor("yout", [B, out_steps, F], F32, kind="ExternalOutput").ap()

    xin_f = xin.rearrange("b t f -> b (t f)")
    yout_f = yout.rearrange("b s f -> b (s f)")

    n_steps = T + out_steps - 1          # total lstm steps (h(T-1+j) -> pred_j)

    with tile.TileContext(nc) as tc:
        import contextlib

        with contextlib.ExitStack() as ctx:
            wpool = ctx.enter_context(tc.tile_pool(name="wpool", bufs=1))
            dpool = ctx.enter_context(tc.tile_pool(name="dpool", bufs=8))
            xspool = ctx.enter_context(tc.tile_pool(name="xspool", bufs=WIN + 2))
            gpool = ctx.enter_context(tc.tile_pool(name="gpool", bufs=3))
            hpool = ctx.enter_context(tc.tile_pool(name="hpool", bufs=3))
            pspool = ctx.enter_context(tc.tile_pool(name="pspool", bufs=2))
            zpool = ctx.enter_context(tc.tile_pool(name="zpool", bufs=3, space="PSUM"))
            upool = ctx.enter_context(tc.tile_pool(name="upool", bufs=2, space="PSUM"))

            # ---- constants / weights ----
            wext = wpool.tile([F + 1, U4], BF16)
            nc.sync.dma_start(wext[:], wext_d[:])
            u2 = wpool.tile([128, 2 * U4], BF16)
            nc.sync.dma_start(u2[:], u2_d[:])
            ud2 = wpool.tile([128, 2 * U4], BF16)
            nc.sync.dma_start(ud2[:], ud2_d[:])
            bdec = wpool.tile([1, U4], BF16)
            nc.sync.dma_start(bdec[:], bdec_d[:])
            wdd = wpool.tile([128, 2 * F], BF16)
            nc.sync.dma_start(wdd[:], wdd_d[:])
            bdrow = wpool.tile([1, F], BF16)
            nc.sync.dma_start(bdrow[:], bdrow_d[:])
            ident = wpool.tile([128, 128], F32)
            nc.sync.dma_start(ident[:], ident_d[:])
            identb = wpool.tile([128, 128], BF16)
            nc.gpsimd.tensor_copy(identb[:], ident[:])
            ones = wpool.tile([1, B], BF16)
            nc.gpsimd.memset(ones[:], 1.0)

            dts = {}      # pair idx -> list of 4 dt tiles [128,128]
            xss = {}      # step -> xs tile [65, B] bf16 (x^T + ones row)

            def load_pair(p):
                tiles = []
                for bc in range(NB):
                    dt = dpool.tile([128, 128], F32, tag="dt", name=f"dt{p}_{bc}")
                    nc.sync.dma_start(
                        dt[:], xin_f[128 * bc : 128 * (bc + 1), 128 * p : 128 * (p + 1)]
                    )
                    db = dpool.tile([128, 128], BF16, tag="db", name=f"db{p}_{bc}")
                    nc.gpsimd.tensor_copy(db[:], dt[:])
                    tiles.append(db)
                dts[p] = tiles

            def stage_step(s):
                """Transpose x_s into xs[s] = [F+1, B] bf16 (ones row at F)."""
                p, half = s // 2, s % 2
                st = upool.tile([F, B], BF16, tag="u", name=f"st{s}")
                for bc in range(NB):
                    nc.tensor.transpose(
                        st[:, 128 * bc : 128 * (bc + 1)],
                        dts[p][bc][:, F * half : F * (half + 1)],
                        identb[:],
                    )
                xs = xspool.tile([F + 1, B], BF16, tag="xs", name=f"xs{s}")
                nc.gpsimd.memset(xs[F : F + 1, :], 1.0)
                nc.vector.tensor_copy(xs[0:F, :], st[:])
                xss[s] = xs
                if half == 1:
                    del dts[p]

            # gate order on PE / Act: f, i, g, o
            GATES = (G_F, G_I, G_G, G_O)
            zt = {}       # (step, gate) -> z tile [128, 2B] psum

            def mcol(q, ch):
                return 256 * q + 128 * ch

            def emit_zstart(t, q):
                """Allocate z tile for (t, q) and emit its bias/x matmul(s)."""
                zq = zpool.tile([128, B2], F32, tag="z", name=f"z{t}_{q}")
                zt[(t, q)] = zq
                last = t == 0       # step 0 has no recurrent term
                if t < T:
                    for ch in (0, 1):
                        nc.tensor.matmul(
                            zq[:, B * ch : B * (ch + 1)],
                            wext[:, mcol(q, ch) : mcol(q, ch) + 128],
                            xss[t][:],
                            start=True,
                            stop=last,
                        )
                else:
                    for ch in (0, 1):
                        nc.tensor.matmul(
                            zq[:, B * ch : B * (ch + 1)],
                            bdec[:, mcol(q, ch) : mcol(q, ch) + 128],
                            ones[:],
                            start=True,
                            stop=False,
                        )
                return zq

            def emit_kmms(t, q, h_prev):
                """Recurrent matmuls for gate q of step t.  For the f gate the
                ch0 output column is finished first so sig(f0) can start before
                h1 of the previous step is even ready."""
                uw = u2 if t < T else ud2
                zq = zt[(t, q)]
                for k, ch in ((0, 0), (0, 1), (1, 0), (1, 1)):
                    nc.tensor.matmul(
                        zq[:, B * ch : B * (ch + 1)],
                        uw[:, U4 * k + mcol(q, ch) : U4 * k + mcol(q, ch) + 128],
                        h_prev[:, B * k : B * (k + 1)],
                        start=False,
                        stop=(k == 1),
                    )

            def emit_act_dve(t, c_prev):
                """Gate activations + cell update for step t. Returns (h, c)."""
                si = gpool.tile([128, B2], BF16, tag="si", name=f"si{t}")
                tg = gpool.tile([128, B2], BF16, tag="tg", name=f"tg{t}")
                so = gpool.tile([128, B2], BF16, tag="so", name=f"so{t}")
                th = gpool.tile([128, B2], BF16, tag="th", name=f"th{t}")
                c_t = gpool.tile([128, B2], F32, tag="c", name=f"c{t}")
                h_t = hpool.tile([128, B2], BF16, tag="h", name=f"h{t}")
                if c_prev is not None:
                    sf = gpool.tile([128, B2], BF16, tag="sf", name=f"sf{t}")
                    t1 = gpool.tile([128, B2], BF16, tag="t1", name=f"t1_{t}")
                    fc = gpool.tile([128, B2], F32, tag="fc", name=f"fc{t}")

                s0 = slice(0, B)
                s1 = slice(B, B2)
                if c_prev is not None:
                    nc.scalar.activation(sf[:], zt.pop((t, G_F))[:], AF.Sigmoid)
                else:
                    zt.pop((t, G_F))   # f gate multiplies c=0: skip
                nc.scalar.activation(si[:], zt.pop((t, G_I))[:], AF.Sigmoid)
                nc.scalar.activation(tg[:], zt.pop((t, G_G))[:], AF.Tanh)
                nc.scalar.activation(so[:], zt.pop((t, G_O))[:], AF.Sigmoid)
                if c_prev is not None:
                    nc.vector.tensor_mul(fc[:, s0], sf[:, s0], c_prev[:, s0])
                    nc.vector.tensor_mul(fc[:, s1], sf[:, s1], c_prev[:, s1])
                    nc.vector.tensor_mul(t1[:, s0], si[:, s0], tg[:, s0])
                    nc.vector.tensor_add(c_t[:, s0], fc[:, s0], t1[:, s0])
                    nc.vector.tensor_mul(t1[:, s1], si[:, s1], tg[:, s1])
                    nc.vector.tensor_add(c_t[:, s1], fc[:, s1], t1[:, s1])
                else:
                    nc.vector.tensor_mul(c_t[:, s0], si[:, s0], tg[:, s0])
                    nc.vector.tensor_mul(c_t[:, s1], si[:, s1], tg[:, s1])
                nc.scalar.activation(th[:, s0], c_t[:, s0], AF.Tanh)
                nc.vector.tensor_mul(h_t[:, s0], so[:, s0], th[:, s0])
                nc.scalar.activation(th[:, s1], c_t[:, s1], AF.Tanh)
                nc.vector.tensor_mul(h_t[:, s1], so[:, s1], th[:, s1])
                return h_t, c_t

            preds = {}    # j -> ps tile [F, B] f32 (pred_j^T, bias included)

            def emit_pred(j, h_t):
                """pred_j^T = Wd^T h + bd -> SBUF [F, B] via PE + Pool only."""
                pp = upool.tile([F, B], F32, tag="u", name=f"pp{j}")
                nc.tensor.matmul(pp[:], bdrow[:], ones[:], start=True, stop=False)
                nc.tensor.matmul(pp[:], wdd[:, 0:F], h_t[:, 0:B], start=False, stop=False)
                nc.tensor.matmul(
                    pp[:], wdd[:, F : 2 * F], h_t[:, B:B2], start=False, stop=True
                )
                ps = pspool.tile([F, B], F32, tag="ps", name=f"ps{j}")
                nc.vector.tensor_copy(ps[:], pp[:])
                preds[j] = ps

            def emit_out(j):
                """Transpose pred_j to [batch, feat] and DMA straight from PSUM."""
                ps = preds.pop(j)
                ot = upool.tile([128, NB * F], F32, tag="u", name=f"ot{j}")
                for bc in range(NB):
                    nc.tensor.transpose(
                        ot[:, F * bc : F * (bc + 1)],
                        ps[:, 128 * bc : 128 * (bc + 1)],
                        ident[0:F, 0:F],
                    )
                osb = pspool.tile([128, NB * F], F32, tag="ob", name=f"ob{j}")
                nc.vector.tensor_copy(osb[:], ot[:])
                for bc in range(NB):
                    nc.sync.dma_start(
                        yout_f[128 * bc : 128 * (bc + 1), F * j : F * (j + 1)],
                        osb[:, F * bc : F * bc + F],
                    )

            # ================= prologue =================
            for p in range((WIN + 1) // 2 + 1):
                load_pair(p)
            for s in range(WIN):
                stage_step(s)
            for q in GATES:
                emit_zstart(0, q)

            # ================= main loop =================
            h_t = c_t = None
            hs = {}
            for t in range(n_steps):
                h_prev, c_prev = h_t, c_t
                if t > 0:
                    # k-chunk matmuls for f, i, g; o's z-start (slot frees after
                    # sig(f) of this step) then o's k-matmuls.
                    emit_kmms(t, G_F, h_prev)
                    emit_kmms(t, G_I, h_prev)
                    emit_kmms(t, G_G, h_prev)
                    emit_zstart(t, G_O)
                    emit_kmms(t, G_O, h_prev)
                # 2-step-lagged pred: matmuls run right after the k-wave, so
                # the pred PSUM->SBUF copy is ready late (DVE idle window)
                if t - 1 >= T:
                    emit_pred(t - 1 - T, hs.pop(t - 2))


                # PE filler work for the recurrence tail: input staging, next
                # step's bias/x matmuls, then the decode output path (after
                # the x matmuls so its PSUM->SBUF copy lands in the DVE's
                # idle window instead of delaying the cell-update chain).
                s = t + WIN
                if s < T:
                    if s % 2 == 0 and s // 2 + 1 < (T + 1) // 2:
                        load_pair(s // 2 + 1)
                    stage_step(s)
                if t > T and t - T >= 2:
                    emit_out(t - T - 2)
                if t + 1 < n_steps:
                    emit_zstart(t + 1, G_F)
                    emit_zstart(t + 1, G_I)
                    emit_zstart(t + 1, G_G)

                h_t, c_t = emit_act_dve(t, c_prev)
                if t >= T - 1:
                    hs[t] = h_t

            # ================= epilogue =================
            for j in (out_steps - 2, out_steps - 1):
                emit_pred(j, hs.pop(j + T - 1))
            for j in range(out_steps - 4, out_steps):
                if j in preds:
                    emit_out(j)

    nc.compile()
    return nc


_CACHE = {}


def _get_program(key):
    if key not in _CACHE:
        _CACHE[key] = build_program(*key)
    return _CACHE[key]


def _host_prep(W, Uk, b, Wd, bd):
    bf = mybir.dt.np(BF16)
    W64 = W.astype(np.float64)
    Ud = (Uk.astype(np.float64) + Wd.astype(np.float64) @ W64).astype(np.float32)
    bdec = (b.astype(np.float64) + bd.astype(np.float64) @ W64).astype(np.float32)
    wext = np.concatenate([W, b.reshape(1, -1)], axis=0)          # [65, 1024]
    u2 = np.concatenate([Uk[0:128], Uk[128:256]], axis=1)         # [128, 2048]
    ud2 = np.concatenate([Ud[0:128], Ud[128:256]], axis=1)
    wdd = np.concatenate([Wd[0:128], Wd[128:256]], axis=1)        # [128, 128]
    ident = np.eye(128, dtype=np.float32)
    return {
        "wext": wext.astype(bf),
        "u2": u2.astype(bf),
        "ud2": ud2.astype(bf),
        "bdec": np.ascontiguousarray(bdec.reshape(1, -1)).astype(bf),
        "wdd": wdd.astype(bf),
        "bdrow": np.ascontiguousarray(bd.reshape(1, -1)).astype(bf),
        "ident": ident,
    }


def kernel(inputs, W, U, b, Wd, bd, out_steps):
    inputs = np.asarray(inputs, dtype=np.float32)
    W = np.asarray(W, dtype=np.float32)
    U_ = np.asarray(U, dtype=np.float32)
    b_ = np.asarray(b, dtype=np.float32)
    Wd = np.asarray(Wd, dtype=np.float32)
    bd = np.asarray(bd, dtype=np.float32)
    out_steps = int(out_steps)

    B_full, T, _ = inputs.shape
    assert B_full % N_CORES == 0
    Bc = B_full // N_CORES

    nc = _get_program((Bc, T, out_steps))
    shared = _host_prep(W, U_, b_, Wd, bd)
    in_maps = [
        {"xin": np.ascontiguousarray(inputs[i * Bc : (i + 1) * Bc]), **shared}
        for i in range(N_CORES)
    ]
    res = bass_utils.run_bass_kernel_spmd(nc, in_maps, core_ids=list(range(N_CORES)))
    out = np.concatenate([res.results[i]["yout"] for i in range(N_CORES)], axis=0)
    return out


# revision 42
# speedup vs baseline: 98.8656x; 1.0139x over previous
"""LSTM warmup + autoregressive decode kernel for Trainium2 (Bass/Tile).

Reference computation (per batch row):
  h,c = 0
  for t in range(T):  h,c = LSTMstep(x_t)        # warmup over input seq
  pred0 = h @ Wd + bd
  for d in range(out_steps-1): h,c = LSTMstep(pred_d); pred_{d+1} = h@Wd+bd
  out[b, s, f] = pred_s

Data-parallel over 8 NeuronCores (B=4096 -> 512/core).  On-chip layout is
transposed (partitions = unit/feature index, free dim = batch) so the
recurrence h^T feeds the next step's matmuls with no per-step transposes.

Key optimizations over the v1 baseline (the recurrence is latency-bound on
the single Activation engine -- 95 sequential steps, each needing 4 gate
activations + tanh(c)):
  * per-gate z tiles are [128, 2B] (both unit-halves), so each gate needs ONE
    activation instruction per step (6 total incl. split tanh(c)) instead of 10.
  * gate biases are folded into the matmuls (ones-row appended to x^T /
    K=1 bias matmul in decode), which is what makes the merged per-gate
    activation legal (bias would otherwise differ across the two unit-halves
    on the same partition).
  * all matmuls run in bf16 (1 cycle/row on the PE at any width); gate values
    and h are bf16 (2x DVE throughput on i*g and o*tanh(c)); c stays fp32.
  * the dense/output path (pred = h@Wd+bd, transpose, DMA) is done entirely
    with PE matmuls + Pool-engine copies + direct PSUM->HBM DMA: zero
    Activation or Vector engine time.
  * autoregressive decode is algebraically fused: Ud = U + Wd@W, so each
    decode step is a single K=256 recurrence with no pred -> x round trip.
  * PE instruction order pipelines next-step x-projections and input
    transposes into the recurrence tail so the PE never blocks the chain.
"""

import sys

for _p in ("/opt/trn_rl_repo", "/root/.axon_site/_ro/trn_rl_repo"):
    if _p not in sys.path:
        sys.path.insert(0, _p)

import numpy as np

import concourse.bacc as bacc
import concourse.mybir as mybir
import concourse.tile as tile
from concourse import bass_utils

F32 = mybir.dt.float32
BF16 = mybir.dt.bfloat16
AF = mybir.ActivationFunctionType

N_CORES = 8
F = 64          # input/output feature dim
U = 256         # lstm units
U4 = 4 * U      # gate rows
# gate order in the 1024-col z layout (keras order i,f,g,o)
G_I, G_F, G_G, G_O = 0, 1, 2, 3


def build_program(B, T, out_steps):
    """Single-core SPMD program for a batch shard of size B (=512)."""
    assert B % 128 == 0
    NB = B // 128
    B2 = 2 * B
    WIN = 10            # x^T staging lookahead (steps)

    nc = bacc.Bacc("TRN2", target_bir_lowering=False, debug=False, num_devices=1)

    xin = nc.dram_tensor("xin", [B, T, F], F32, kind="ExternalInput").ap()
    wext_d = nc.dram_tensor("wext", [F + 1, U4], BF16, kind="ExternalInput").ap()
    u2_d = nc.dram_tensor("u2", [128, 2 * U4], BF16, kind="ExternalInput").ap()
    ud2_d = nc.dram_tensor("ud2", [128, 2 * U4], BF16, kind="ExternalInput").ap()
    bdec_d = nc.dram_tensor("bdec", [1, U4], BF16, kind="ExternalInput").ap()
    wdd_d = nc.dram_tensor("wdd", [128, 2 * F], BF16, kind="ExternalInput").ap()
    bdrow_d = nc.dram_tensor("bdrow", [1, F], BF16, kind="ExternalInput").ap()
    ident_d = nc.dram_tensor("ident", [128, 128], F32, kind="ExternalInput").ap()
    yout = nc.dram_tensor("yout", [B, out_steps, F], F32, kind="ExternalOutput").ap()

    xin_f = xin.rearrange("b t f -> b (t f)")
    yout_f = yout.rearrange("b s f -> b (s f)")

    n_steps = T + out_steps - 1          # total lstm steps (h(T-1+j) -> pred_j)

    with tile.TileContext(nc) as tc:
        import contextlib

        with contextlib.ExitStack() as ctx:
            wpool = ctx.enter_context(tc.tile_pool(name="wpool", bufs=1))
            dpool = ctx.enter_context(tc.tile_pool(name="dpool", bufs=8))
            xspool = ctx.enter_context(tc.tile_pool(name="xspool", bufs=WIN + 2))
            gpool = ctx.enter_context(tc.tile_pool(name="gpool", bufs=3))
            hpool = ctx.enter_context(tc.tile_pool(name="hpool", bufs=3))
            pspool = ctx.enter_context(tc.tile_pool(name="pspool", bufs=2))
            zpool = ctx.enter_context(tc.tile_pool(name="zpool", bufs=3, space="PSUM"))
            upool = ctx.enter_context(tc.tile_pool(name="upool", bufs=2, space="PSUM"))

            # ---- constants / weights ----
            wext = wpool.tile([F + 1, U4], BF16)
            nc.sync.dma_start(wext[:], wext_d[:])
            u2 = wpool.tile([128, 2 * U4], BF16)
            nc.sync.dma_start(u2[:], u2_d[:])
            ud2 = wpool.tile([128, 2 * U4], BF16)
            nc.sync.dma_start(ud2[:], ud2_d[:])
            bdec = wpool.tile([1, U4], BF16)
            nc.sync.dma_start(bdec[:], bdec_d[:])
            wdd = wpool.tile([128, 2 * F], BF16)
            nc.sync.dma_start(wdd[:], wdd_d[:])
            bdrow = wpool.tile([1, F], BF16)
            nc.sync.dma_start(bdrow[:], bdrow_d[:])
            ident = wpool.tile([128, 128], F32)
            nc.sync.dma_start(ident[:], ident_d[:])
            identb = wpool.tile([128, 128], BF16)
            nc.gpsimd.tensor_copy(identb[:], ident[:])
            ones = wpool.tile([1, B], BF16)
            nc.gpsimd.memset(ones[:], 1.0)

            dts = {}      # pair idx -> list of 4 dt tiles [128,128]
            xss = {}      # step -> xs tile [65, B] bf16 (x^T + ones row)

            def load_pair(p):
                tiles = []
                for bc in range(NB):
                    dt = dpool.tile([128, 128], F32, tag="dt", name=f"dt{p}_{bc}")
                    nc.sync.dma_start(
                        dt[:], xin_f[128 * bc : 128 * (bc + 1), 128 * p : 128 * (p + 1)]
                    )
                    db = dpool.tile([128, 128], BF16, tag="db", name=f"db{p}_{bc}")
                    nc.gpsimd.tensor_copy(db[:], dt[:])
                    tiles.append(db)
                dts[p] = tiles

            def stage_step(s):
                """Transpose x_s into xs[s] = [F+1, B] bf16 (ones row at F)."""
                p, half = s // 2, s % 2
                st = upool.tile([F, B], BF16, tag="u", name=f"st{s}")
                for bc in range(NB):
                    nc.tensor.transpose(
                        st[:, 128 * bc : 128 * (bc + 1)],
                        dts[p][bc][:, F * half : F * (half + 1)],
                        identb[:],
                    )
                xs = xspool.tile([F + 1, B], BF16, tag="xs", name=f"xs{s}")
                nc.gpsimd.memset(xs[F : F + 1, :], 1.0)
                nc.vector.tensor_copy(xs[0:F, :], st[:])
                xss[s] = xs
                if half == 1:
                    del dts[p]

            # gate order on PE / Act: f, i, g, o
            GATES = (G_F, G_I, G_G, G_O)
            zt = {}       # (step, gate) -> z tile [128, 2B] psum

            def mcol(q, ch):
                return 256 * q + 128 * ch

            def emit_zstart(t, q):
                """Allocate z tile for (t, q) and emit its bias/x matmul(s)."""
                zq = zpool.tile([128, B2], F32, tag="z", name=f"z{t}_{q}")
                zt[(t, q)] = zq
                last = t == 0       # step 0 has no recurrent term
                if t < T:
                    for ch in (0, 1):
                        nc.tensor.matmul(
                            zq[:, B * ch : B * (ch + 1)],
                            wext[:, mcol(q, ch) : mcol(q, ch) + 128],
                            xss[t][:],
                            start=True,
                            stop=last,
                        )
                else:
                    for ch in (0, 1):
                        nc.tensor.matmul(
                            zq[:, B * ch : B * (ch + 1)],
                            bdec[:, mcol(q, ch) : mcol(q, ch) + 128],
                            ones[:],
                            start=True,
                            stop=False,
                        )
                return zq

            def emit_kmms(t, q, h_prev):
                """Recurrent matmuls for gate q of step t.  For the f gate the
                ch0 output column is finished first so sig(f0) can start before
                h1 of the previous step is even ready."""
                uw = u2 if t < T else ud2
                zq = zt[(t, q)]
                for k, ch in ((0, 0), (0, 1), (1, 0), (1, 1)):
                    nc.tensor.matmul(
                        zq[:, B * ch : B * (ch + 1)],
                        uw[:, U4 * k + mcol(q, ch) : U4 * k + mcol(q, ch) + 128],
                        h_prev[:, B * k : B * (k + 1)],
                        start=False,
                        stop=(k == 1),
                    )

            def emit_act_dve(t, c_prev):
                """Gate activations + cell update for step t. Returns (h, c)."""
                si = gpool.tile([128, B2], BF16, tag="si", name=f"si{t}")
                tg = gpool.tile([128, B2], BF16, tag="tg", name=f"tg{t}")
                so = gpool.tile([128, B2], BF16, tag="so", name=f"so{t}")
                th = gpool.tile([128, B2], BF16, tag="th", name=f"th{t}")
                c_t = gpool.tile([128, B2], F32, tag="c", name=f"c{t}")
                h_t = hpool.tile([128, B2], BF16, tag="h", name=f"h{t}")
                if c_prev is not None:
                    sf = gpool.tile([128, B2], BF16, tag="sf", name=f"sf{t}")
                    t1 = gpool.tile([128, B2], BF16, tag="t1", name=f"t1_{t}")
                    fc = gpool.tile([128, B2], F32, tag="fc", name=f"fc{t}")

                s0 = slice(0, B)
                s1 = slice(B, B2)
                if c_prev is not None:
                    nc.scalar.activation(sf[:], zt.pop((t, G_F))[:], AF.Sigmoid)
                else:
                    zt.pop((t, G_F))   # f gate multiplies c=0: skip
                nc.scalar.activation(si[:], zt.pop((t, G_I))[:], AF.Sigmoid)
                nc.scalar.activation(tg[:], zt.pop((t, G_G))[:], AF.Tanh)
                nc.scalar.activation(so[:], zt.pop((t, G_O))[:], AF.Sigmoid)
                if c_prev is not None:
                    nc.vector.tensor_mul(fc[:, s0], sf[:, s0], c_prev[:, s0])
                    nc.vector.tensor_mul(fc[:, s1], sf[:, s1], c_prev[:, s1])
                    nc.vector.tensor_mul(t1[:, s0], si[:, s0], tg[:, s0])
                    nc.gpsimd.tensor_mul(t1[:, s1], si[:, s1], tg[:, s1])
                    nc.vector.tensor_add(c_t[:, s0], fc[:, s0], t1[:, s0])
                    nc.vector.tensor_add(c_t[:, s1], fc[:, s1], t1[:, s1])
                else:
                    nc.vector.tensor_mul(c_t[:, s0], si[:, s0], tg[:, s0])
                    nc.vector.tensor_mul(c_t[:, s1], si[:, s1], tg[:, s1])
                nc.scalar.activation(th[:, s0], c_t[:, s0], AF.Tanh)
                nc.vector.tensor_mul(h_t[:, s0], so[:, s0], th[:, s0])
                nc.scalar.activation(th[:, s1], c_t[:, s1], AF.Tanh)
                nc.vector.tensor_mul(h_t[:, s1], so[:, s1], th[:, s1])
                return h_t, c_t

            preds = {}    # j -> ps tile [F, B] f32 (pred_j^T, bias included)

            def emit_pred(j, h_t):
                """pred_j^T = Wd^T h + bd -> SBUF [F, B] via PE + Pool only."""
                pp = upool.tile([F, B], F32, tag="u", name=f"pp{j}")
                nc.tensor.matmul(pp[:], bdrow[:], ones[:], start=True, stop=False)
                nc.tensor.matmul(pp[:], wdd[:, 0:F], h_t[:, 0:B], start=False, stop=False)
                nc.tensor.matmul(
                    pp[:], wdd[:, F : 2 * F], h_t[:, B:B2], start=False, stop=True
                )
                ps = pspool.tile([F, B], F32, tag="ps", name=f"ps{j}")
                nc.vector.tensor_copy(ps[:], pp[:])
                preds[j] = ps

            def emit_out(j):
                """Transpose pred_j to [batch, feat] and DMA straight from PSUM."""
                ps = preds.pop(j)
                ot = upool.tile([128, NB * F], F32, tag="u", name=f"ot{j}")
                for bc in range(NB):
                    nc.tensor.transpose(
                        ot[:, F * bc : F * (bc + 1)],
                        ps[:, 128 * bc : 128 * (bc + 1)],
                        ident[0:F, 0:F],
                    )
                osb = pspool.tile([128, NB * F], F32, tag="ob", name=f"ob{j}")
                nc.vector.tensor_copy(osb[:], ot[:])
                for bc in range(NB):
                    nc.sync.dma_start(
                        yout_f[128 * bc : 128 * (bc + 1), F * j : F * (j + 1)],
                        osb[:, F * bc : F * bc + F],
                    )

            # ================= prologue =================
            for p in range((WIN + 1) // 2 + 1):
                load_pair(p)
            for s in range(WIN):
                stage_step(s)
            for q in GATES:
                emit_zstart(0, q)

            # ================= main loop =================
            h_t = c_t = None
            hs = {}
            for t in range(n_steps):
                h_prev, c_prev = h_t, c_t
                if t > 0:
                    # k-chunk matmuls for f, i, g; o's z-start (slot frees after
                    # sig(f) of this step) then o's k-matmuls.
                    emit_kmms(t, G_F, h_prev)
                    emit_kmms(t, G_I, h_prev)
                    emit_kmms(t, G_G, h_prev)
                    emit_zstart(t, G_O)
                    emit_kmms(t, G_O, h_prev)
                # 2-step-lagged pred: matmuls run right after the k-wave, so
                # the pred PSUM->SBUF copy is ready late (DVE idle window)
                if t - 1 >= T:
                    emit_pred(t - 1 - T, hs.pop(t - 2))


                # PE filler work for the recurrence tail: input staging, next
                # step's bias/x matmuls, then the decode output path (after
                # the x matmuls so its PSUM->SBUF copy lands in the DVE's
                # idle window instead of delaying the cell-update chain).
                s = t + WIN
                if s < T:
                    if s % 2 == 0 and s // 2 + 1 < (T + 1) // 2:
                        load_pair(s // 2 + 1)
                    stage_step(s)
                if t > T and t - T >= 2:
                    emit_out(t - T - 2)
                if t + 1 < n_steps:
                    emit_zstart(t + 1, G_F)
                    emit_zstart(t + 1, G_I)
                    emit_zstart(t + 1, G_G)

                h_t, c_t = emit_act_dve(t, c_prev)
                if t >= T - 1:
                    hs[t] = h_t

            # ================= epilogue =================
            for j in (out_steps - 2, out_steps - 1):
                emit_pred(j, hs.pop(j + T - 1))
            for j in range(out_steps - 4, out_steps):
                if j in preds:
                    emit_out(j)

    nc.compile()
    return nc


_CACHE = {}


def _get_program(key):
    if key not in _CACHE:
        _CACHE[key] = build_program(*key)
    return _CACHE[key]


def _host_prep(W, Uk, b, Wd, bd):
    bf = mybir.dt.np(BF16)
    W64 = W.astype(np.float64)
    Ud = (Uk.astype(np.float64) + Wd.astype(np.float64) @ W64).astype(np.float32)
    bdec = (b.astype(np.float64) + bd.astype(np.float64) @ W64).astype(np.float32)
    wext = np.concatenate([W, b.reshape(1, -1)], axis=0)          # [65, 1024]
    u2 = np.concatenate([Uk[0:128], Uk[128:256]], axis=1)         # [128, 2048]
    ud2 = np.concatenate([Ud[0:128], Ud[128:256]], axis=1)
    wdd = np.concatenate([Wd[0:128], Wd[128:256]], axis=1)        # [128, 128]
    ident = np.eye(128, dtype=np.float32)
    return {
        "wext": wext.astype(bf),
        "u2": u2.astype(bf),
        "ud2": ud2.astype(bf),
        "bdec": np.ascontiguousarray(bdec.reshape(1, -1)).astype(bf),
        "wdd": wdd.astype(bf),
        "bdrow": np.ascontiguousarray(bd.reshape(1, -1)).astype(bf),
        "ident": ident,
    }


def kernel(inputs, W, U, b, Wd, bd, out_steps):
    inputs = np.asarray(inputs, dtype=np.float32)
    W = np.asarray(W, dtype=np.float32)
    U_ = np.asarray(U, dtype=np.float32)
    b_ = np.asarray(b, dtype=np.float32)
    Wd = np.asarray(Wd, dtype=np.float32)
    bd = np.asarray(bd, dtype=np.float32)
    out_steps = int(out_steps)

    B_full, T, _ = inputs.shape
    assert B_full % N_CORES == 0
    Bc = B_full // N_CORES

    nc = _get_program((Bc, T, out_steps))
    shared = _host_prep(W, U_, b_, Wd, bd)
    in_maps = [
        {"xin": np.ascontiguousarray(inputs[i * Bc : (i + 1) * Bc]), **shared}
        for i in range(N_CORES)
    ]
    res = bass_utils.run_bass_kernel_spmd(nc, in_maps, core_ids=list(range(N_CORES)))
    out = np.concatenate([res.results[i]["yout"] for i in range(N_CORES)], axis=0)
    return out


# revision 48
# speedup vs baseline: 101.1751x; 1.0234x over previous
"""LSTM warmup + autoregressive decode kernel for Trainium2 (Bass/Tile).

Reference computation (per batch row):
  h,c = 0
  for t in range(T):  h,c = LSTMstep(x_t)        # warmup over input seq
  pred0 = h @ Wd + bd
  for d in range(out_steps-1): h,c = LSTMstep(pred_d); pred_{d+1} = h@Wd+bd
  out[b, s, f] = pred_s

Data-parallel over 8 NeuronCores (B=4096 -> 512/core).  On-chip layout is
transposed (partitions = unit/feature index, free dim = batch) so the
recurrence h^T feeds the next step's matmuls with no per-step transposes.

Key optimizations over the v1 baseline (the recurrence is latency-bound on
the single Activation engine -- 95 sequential steps, each needing 4 gate
activations + tanh(c)):
  * per-gate z tiles are [128, 2B] (both unit-halves), so each gate needs ONE
    activation instruction per step (6 total incl. split tanh(c)) instead of 10.
  * gate biases are folded into the matmuls (ones-row appended to x^T /
    K=1 bias matmul in decode), which is what makes the merged per-gate
    activation legal (bias would otherwise differ across the two unit-halves
    on the same partition).
  * all matmuls run in bf16 (1 cycle/row on the PE at any width); gate values
    and h are bf16 (2x DVE throughput on i*g and o*tanh(c)); c stays fp32.
  * the dense/output path (pred = h@Wd+bd, transpose, DMA) is done entirely
    with PE matmuls + Pool-engine copies + direct PSUM->HBM DMA: zero
    Activation or Vector engine time.
  * autoregressive decode is algebraically fused: Ud = U + Wd@W, so each
    decode step is a single K=256 recurrence with no pred -> x round trip.
  * PE instruction order pipelines next-step x-projections and input
    transposes into the recurrence tail so the PE never blocks the chain.
"""

import sys

for _p in ("/opt/trn_rl_repo", "/root/.axon_site/_ro/trn_rl_repo"):
    if _p not in sys.path:
        sys.path.insert(0, _p)

import numpy as np

import concourse.bacc as bacc
import concourse.mybir as mybir
import concourse.tile as tile
from concourse import bass_utils

F32 = mybir.dt.float32
BF16 = mybir.dt.bfloat16
FP16 = mybir.dt.float16
AF = mybir.ActivationFunctionType

N_CORES = 8
F = 64          # input/output feature dim
U = 256         # lstm units
U4 = 4 * U      # gate rows
# gate order in the 1024-col z layout (keras order i,f,g,o)
G_I, G_F, G_G, G_O = 0, 1, 2, 3


def build_program(B, T, out_steps):
    """Single-core SPMD program for a batch shard of size B (=512)."""
    assert B % 128 == 0
    NB = B // 128
    B2 = 2 * B
    WIN = 10            # x^T staging lookahead (steps)

    nc = bacc.Bacc("TRN2", target_bir_lowering=False, debug=False, num_devices=1)

    xin = nc.dram_tensor("xin", [B, T, F], F32, kind="ExternalInput").ap()
    wext_d = nc.dram_tensor("wext", [F + 1, U4], BF16, kind="ExternalInput").ap()
    u2_d = nc.dram_tensor("u2", [128, 2 * U4], BF16, kind="ExternalInput").ap()
    ud2_d = nc.dram_tensor("ud2", [128, 2 * U4], BF16, kind="ExternalInput").ap()
    bdec_d = nc.dram_tensor("bdec", [1, U4], BF16, kind="ExternalInput").ap()
    wdd_d = nc.dram_tensor("wdd", [128, 2 * F], BF16, kind="ExternalInput").ap()
    bdrow_d = nc.dram_tensor("bdrow", [1, F], BF16, kind="ExternalInput").ap()
    ident_d = nc.dram_tensor("ident", [128, 128], F32, kind="ExternalInput").ap()
    yout = nc.dram_tensor("yout", [B, out_steps, F], F32, kind="ExternalOutput").ap()

    xin_f = xin.rearrange("b t f -> b (t f)")
    yout_f = yout.rearrange("b s f -> b (s f)")

    n_steps = T + out_steps - 1          # total lstm steps (h(T-1+j) -> pred_j)

    with tile.TileContext(nc) as tc:
        import contextlib

        with contextlib.ExitStack() as ctx:
            wpool = ctx.enter_context(tc.tile_pool(name="wpool", bufs=1))
            dpool = ctx.enter_context(tc.tile_pool(name="dpool", bufs=8))
            xspool = ctx.enter_context(tc.tile_pool(name="xspool", bufs=WIN + 2))
            gpool = ctx.enter_context(tc.tile_pool(name="gpool", bufs=3))
            hpool = ctx.enter_context(tc.tile_pool(name="hpool", bufs=3))
            pspool = ctx.enter_context(tc.tile_pool(name="pspool", bufs=2))
            zpool = ctx.enter_context(tc.tile_pool(name="zpool", bufs=3, space="PSUM"))
            upool = ctx.enter_context(tc.tile_pool(name="upool", bufs=2, space="PSUM"))

            # ---- constants / weights ----
            wext = wpool.tile([F + 1, U4], BF16)
            nc.sync.dma_start(wext[:], wext_d[:])
            u2 = wpool.tile([128, 2 * U4], BF16)
            nc.sync.dma_start(u2[:], u2_d[:])
            ud2 = wpool.tile([128, 2 * U4], BF16)
            nc.sync.dma_start(ud2[:], ud2_d[:])
            bdec = wpool.tile([1, U4], BF16)
            nc.sync.dma_start(bdec[:], bdec_d[:])
            wdd = wpool.tile([128, 2 * F], BF16)
            nc.sync.dma_start(wdd[:], wdd_d[:])
            bdrow = wpool.tile([1, F], BF16)
            nc.sync.dma_start(bdrow[:], bdrow_d[:])
            ident = wpool.tile([128, 128], F32)
            nc.sync.dma_start(ident[:], ident_d[:])
            identb = wpool.tile([128, 128], BF16)
            nc.gpsimd.tensor_copy(identb[:], ident[:])
            ones = wpool.tile([1, B], BF16)
            nc.gpsimd.memset(ones[:], 1.0)

            dts = {}      # pair idx -> list of 4 dt tiles [128,128]
            xss = {}      # step -> xs tile [65, B] bf16 (x^T + ones row)

            def load_pair(p):
                tiles = []
                for bc in range(NB):
                    dt = dpool.tile([128, 128], F32, tag="dt", name=f"dt{p}_{bc}")
                    nc.sync.dma_start(
                        dt[:], xin_f[128 * bc : 128 * (bc + 1), 128 * p : 128 * (p + 1)]
                    )
                    db = dpool.tile([128, 128], BF16, tag="db", name=f"db{p}_{bc}")
                    nc.gpsimd.tensor_copy(db[:], dt[:])
                    tiles.append(db)
                dts[p] = tiles

            def stage_step(s):
                """Transpose x_s into xs[s] = [F+1, B] bf16 (ones row at F)."""
                p, half = s // 2, s % 2
                st = upool.tile([F, B], BF16, tag="u", name=f"st{s}")
                for bc in range(NB):
                    nc.tensor.transpose(
                        st[:, 128 * bc : 128 * (bc + 1)],
                        dts[p][bc][:, F * half : F * (half + 1)],
                        identb[:],
                    )
                xs = xspool.tile([F + 1, B], BF16, tag="xs", name=f"xs{s}")
                nc.gpsimd.memset(xs[F : F + 1, :], 1.0)
                nc.vector.tensor_copy(xs[0:F, :], st[:])
                xss[s] = xs
                if half == 1:
                    del dts[p]

            # gate order on PE / Act: f, i, g, o
            GATES = (G_F, G_I, G_G, G_O)
            zt = {}       # (step, gate) -> z tile [128, 2B] psum

            def mcol(q, ch):
                return 256 * q + 128 * ch

            def emit_zstart(t, q):
                """Allocate z tile for (t, q) and emit its bias/x matmul(s)."""
                zq = zpool.tile([128, B2], F32, tag="z", name=f"z{t}_{q}")
                zt[(t, q)] = zq
                last = t == 0       # step 0 has no recurrent term
                if t < T:
                    for ch in (0, 1):
                        nc.tensor.matmul(
                            zq[:, B * ch : B * (ch + 1)],
                            wext[:, mcol(q, ch) : mcol(q, ch) + 128],
                            xss[t][:],
                            start=True,
                            stop=last,
                        )
                else:
                    for ch in (0, 1):
                        nc.tensor.matmul(
                            zq[:, B * ch : B * (ch + 1)],
                            bdec[:, mcol(q, ch) : mcol(q, ch) + 128],
                            ones[:],
                            start=True,
                            stop=False,
                        )
                return zq

            def emit_kmms(t, q, h_prev):
                """Recurrent matmuls for gate q of step t.  For the f gate the
                ch0 output column is finished first so sig(f0) can start before
                h1 of the previous step is even ready."""
                uw = u2 if t < T else ud2
                zq = zt[(t, q)]
                for k, ch in ((0, 0), (0, 1), (1, 0), (1, 1)):
                    nc.tensor.matmul(
                        zq[:, B * ch : B * (ch + 1)],
                        uw[:, U4 * k + mcol(q, ch) : U4 * k + mcol(q, ch) + 128],
                        h_prev[:, B * k : B * (k + 1)],
                        start=False,
                        stop=(k == 1),
                    )

            def emit_act_dve(t, c_prev):
                """Gate activations + cell update for step t. Returns (h, c)."""
                si = gpool.tile([128, B2], BF16, tag="si", name=f"si{t}")
                tg = gpool.tile([128, B2], BF16, tag="tg", name=f"tg{t}")
                so = gpool.tile([128, B2], BF16, tag="so", name=f"so{t}")
                th = gpool.tile([128, B2], BF16, tag="th", name=f"th{t}")
                c_t = gpool.tile([128, B2], FP16, tag="c", name=f"c{t}")
                h_t = hpool.tile([128, B2], BF16, tag="h", name=f"h{t}")
                if c_prev is not None:
                    sf = gpool.tile([128, B2], BF16, tag="sf", name=f"sf{t}")
                    t1 = gpool.tile([128, B2], BF16, tag="t1", name=f"t1_{t}")
                    fc = gpool.tile([128, B2], FP16, tag="fc", name=f"fc{t}")

                s0 = slice(0, B)
                s1 = slice(B, B2)
                if c_prev is not None:
                    nc.scalar.activation(sf[:], zt.pop((t, G_F))[:], AF.Sigmoid)
                else:
                    zt.pop((t, G_F))   # f gate multiplies c=0: skip
                nc.scalar.activation(si[:], zt.pop((t, G_I))[:], AF.Sigmoid)
                nc.scalar.activation(tg[:], zt.pop((t, G_G))[:], AF.Tanh)
                nc.scalar.activation(so[:], zt.pop((t, G_O))[:], AF.Sigmoid)
                if c_prev is not None:
                    nc.vector.tensor_mul(fc[:, s0], sf[:, s0], c_prev[:, s0])
                    nc.vector.tensor_mul(fc[:, s1], sf[:, s1], c_prev[:, s1])
                    nc.vector.tensor_mul(t1[:, s0], si[:, s0], tg[:, s0])
                    nc.gpsimd.tensor_mul(t1[:, s1], si[:, s1], tg[:, s1])
                    nc.vector.tensor_add(c_t[:, s0], fc[:, s0], t1[:, s0])
                    nc.vector.tensor_add(c_t[:, s1], fc[:, s1], t1[:, s1])
                else:
                    nc.vector.tensor_mul(c_t[:, s0], si[:, s0], tg[:, s0])
                    nc.vector.tensor_mul(c_t[:, s1], si[:, s1], tg[:, s1])
                nc.scalar.activation(th[:, s0], c_t[:, s0], AF.Tanh)
                nc.vector.tensor_mul(h_t[:, s0], so[:, s0], th[:, s0])
                nc.scalar.activation(th[:, s1], c_t[:, s1], AF.Tanh)
                nc.vector.tensor_mul(h_t[:, s1], so[:, s1], th[:, s1])
                return h_t, c_t

            preds = {}    # j -> ps tile [F, B] f32 (pred_j^T, bias included)

            def emit_pred(j, h_t):
                """pred_j^T = Wd^T h + bd -> SBUF [F, B] via PE + Pool only."""
                pp = upool.tile([F, B], F32, tag="u", name=f"pp{j}")
                nc.tensor.matmul(pp[:], bdrow[:], ones[:], start=True, stop=False)
                nc.tensor.matmul(pp[:], wdd[:, 0:F], h_t[:, 0:B], start=False, stop=False)
                nc.tensor.matmul(
                    pp[:], wdd[:, F : 2 * F], h_t[:, B:B2], start=False, stop=True
                )
                ps = pspool.tile([F, B], F32, tag="ps", name=f"ps{j}")
                nc.vector.tensor_copy(ps[:], pp[:])
                preds[j] = ps

            def emit_out(j):
                """Transpose pred_j to [batch, feat] and DMA straight from PSUM."""
                ps = preds.pop(j)
                ot = upool.tile([128, NB * F], F32, tag="u", name=f"ot{j}")
                for bc in range(NB):
                    nc.tensor.transpose(
                        ot[:, F * bc : F * (bc + 1)],
                        ps[:, 128 * bc : 128 * (bc + 1)],
                        ident[0:F, 0:F],
                    )
                osb = pspool.tile([128, NB * F], F32, tag="ob", name=f"ob{j}")
                nc.vector.tensor_copy(osb[:], ot[:])
                for bc in range(NB):
                    nc.sync.dma_start(
                        yout_f[128 * bc : 128 * (bc + 1), F * j : F * (j + 1)],
                        osb[:, F * bc : F * bc + F],
                    )

            # ================= prologue =================
            for p in range((WIN + 1) // 2 + 1):
                load_pair(p)
            for s in range(WIN):
                stage_step(s)
            for q in GATES:
                emit_zstart(0, q)

            # ================= main loop =================
            h_t = c_t = None
            hs = {}
            for t in range(n_steps):
                h_prev, c_prev = h_t, c_t
                if t > 0:
                    # k-chunk matmuls for f, i, g; o's z-start (slot frees after
                    # sig(f) of this step) then o's k-matmuls.
                    emit_kmms(t, G_F, h_prev)
                    emit_kmms(t, G_I, h_prev)
                    emit_kmms(t, G_G, h_prev)
                    emit_zstart(t, G_O)
                    emit_kmms(t, G_O, h_prev)
                # 2-step-lagged pred: matmuls run right after the k-wave, so
                # the pred PSUM->SBUF copy is ready late (DVE idle window)
                if t - 1 >= T:
                    emit_pred(t - 1 - T, hs.pop(t - 2))


                # PE filler work for the recurrence tail: input staging, next
                # step's bias/x matmuls, then the decode output path (after
                # the x matmuls so its PSUM->SBUF copy lands in the DVE's
                # idle window instead of delaying the cell-update chain).
                s = t + WIN
                if s < T:
                    if s % 2 == 0 and s // 2 + 1 < (T + 1) // 2:
                        load_pair(s // 2 + 1)
                    stage_step(s)
                if t > T and t - T >= 2:
                    emit_out(t - T - 2)
                if t + 1 < n_steps:
                    emit_zstart(t + 1, G_F)
                    emit_zstart(t + 1, G_I)
                    emit_zstart(t + 1, G_G)

                h_t, c_t = emit_act_dve(t, c_prev)
                if t >= T - 1:
                    hs[t] = h_t

            # ================= epilogue =================
            for j in (out_steps - 2, out_steps - 1):
                emit_pred(j, hs.pop(j + T - 1))
            for j in range(out_steps - 4, out_steps):
                if j in preds:
                    emit_out(j)

    nc.compile()
    return nc


_CACHE = {}


def _get_program(key):
    if key not in _CACHE:
        _CACHE[key] = build_program(*key)
    return _CACHE[key]


def _host_prep(W, Uk, b, Wd, bd):
    bf = mybir.dt.np(BF16)
    W64 = W.astype(np.float64)
    Ud = (Uk.astype(np.float64) + Wd.astype(np.float64) @ W64).astype(np.float32)
    bdec = (b.astype(np.float64) + bd.astype(np.float64) @ W64).astype(np.float32)
    wext = np.concatenate([W, b.reshape(1, -1)], axis=0)          # [65, 1024]
    u2 = np.concatenate([Uk[0:128], Uk[128:256]], axis=1)         # [128, 2048]
    ud2 = np.concatenate([Ud[0:128], Ud[128:256]], axis=1)
    wdd = np.concatenate([Wd[0:128], Wd[128:256]], axis=1)        # [128, 128]
    ident = np.eye(128, dtype=np.float32)
    return {
        "wext": wext.astype(bf),
        "u2": u2.astype(bf),
        "ud2": ud2.astype(bf),
        "bdec": np.ascontiguousarray(bdec.reshape(1, -1)).astype(bf),
        "wdd": wdd.astype(bf),
        "bdrow": np.ascontiguousarray(bd.reshape(1, -1)).astype(bf),
        "ident": ident,
    }


def kernel(inputs, W, U, b, Wd, bd, out_steps):
    inputs = np.asarray(inputs, dtype=np.float32)
    W = np.asarray(W, dtype=np.float32)
    U_ = np.asarray(U, dtype=np.float32)
    b_ = np.asarray(b, dtype=np.float32)
    Wd = np.asarray(Wd, dtype=np.float32)
    bd = np.asarray(bd, dtype=np.float32)
    out_steps = int(out_steps)

    B_full, T, _ = inputs.shape
    assert B_full % N_CORES == 0
    Bc = B_full // N_CORES

    nc = _get_program((Bc, T, out_steps))
    shared = _host_prep(W, U_, b_, Wd, bd)
    in_maps = [
        {"xin": np.ascontiguousarray(inputs[i * Bc : (i + 1) * Bc]), **shared}
        for i in range(N_CORES)
    ]
    res = bass_utils.run_bass_kernel_spmd(nc, in_maps, core_ids=list(range(N_CORES)))
    out = np.concatenate([res.results[i]["yout"] for i in range(N_CORES)], axis=0)
    return out


# revision 58
# speedup vs baseline: 104.2106x; 1.0300x over previous
"""LSTM warmup + autoregressive decode kernel for Trainium2 (Bass/Tile).

Reference computation (per batch row):
  h,c = 0
  for t in range(T):  h,c = LSTMstep(x_t)        # warmup over input seq
  pred0 = h @ Wd + bd
  for d in range(out_steps-1): h,c = LSTMstep(pred_d); pred_{d+1} = h@Wd+bd
  out[b, s, f] = pred_s

Data-parallel over 8 NeuronCores (B=4096 -> 512/core).  On-chip layout is
transposed (partitions = unit/feature index, free dim = batch) so the
recurrence h^T feeds the next step's matmuls with no per-step transposes.

The recurrence is latency-bound: 95 sequential steps, each needing 4 gate
activations + tanh(c) on the single (1.2 GHz) Activation engine, whose
~5.4us/step of table lookups is the hard floor.  Key optimizations over the
v1 baseline (modeled 822us -> 679us):
  * per-gate z tiles are [128, 2B] (both unit-halves), so each gate needs ONE
    activation instruction per step (6 total incl. split tanh(c)) instead of
    10 -- activation instructions pay a fixed ~185ns SBUF-access cost.
  * gate biases are folded into the matmuls (ones-row appended to x^T in
    warmup / K=1 ones-vector bias matmul in decode), which is what makes the
    merged per-gate activation legal (the bias AP is per-partition, but the
    bias differs across the two unit-halves sharing a partition).
  * all matmuls run in bf16 (1 PE cycle/row at any width); gate values and h
    are bf16 and c is fp16, so every cell-update vector op hits the DVE
    2x 2-byte mode; fp16 (not bf16) for c keeps the accumulated cell-state
    rounding noise ~8x lower at the same speed (rel err 4.9e-3 vs 5.7e-3).
  * one half of the i*tanh(g) product is offloaded to the otherwise-idle
    GPSIMD engine, which removes a DVE readiness-order inversion that
    otherwise delays the cell update by ~330ns every step.
  * the dense/output path computes pred = h@Wd+bd DIRECTLY in [batch, feat]
    layout by using h slices as the matmul STATIONARY operand (output free
    dim is only F=64, so each matmul costs ~27ns); bias rides a ones^T x bd
    K=1 matmul.  No transposes, one small DVE copy, zero Activation time.
  * autoregressive decode is algebraically fused: Ud = U + Wd@W, so each
    decode step is a single K=256 recurrence with no pred -> x round trip.
  * PE instruction order feeds the chain first (f,i,g then o k-matmuls) and
    pipelines next-step x-projections and the (bf16) input transposes into
    the recurrence tail so the PE never blocks the chain.
"""

import sys

for _p in ("/opt/trn_rl_repo", "/root/.axon_site/_ro/trn_rl_repo"):
    if _p not in sys.path:
        sys.path.insert(0, _p)

import numpy as np

import concourse.bacc as bacc
import concourse.mybir as mybir
import concourse.tile as tile
from concourse import bass_utils

F32 = mybir.dt.float32
BF16 = mybir.dt.bfloat16
FP16 = mybir.dt.float16
AF = mybir.ActivationFunctionType

N_CORES = 8
F = 64          # input/output feature dim
U = 256         # lstm units
U4 = 4 * U      # gate rows
# gate order in the 1024-col z layout (keras order i,f,g,o)
G_I, G_F, G_G, G_O = 0, 1, 2, 3


def build_program(B, T, out_steps):
    """Single-core SPMD program for a batch shard of size B (=512)."""
    assert B % 128 == 0
    NB = B // 128
    B2 = 2 * B
    WIN = 10            # x^T staging lookahead (steps)

    nc = bacc.Bacc("TRN2", target_bir_lowering=False, debug=False, num_devices=1)

    xin = nc.dram_tensor("xin", [B, T, F], F32, kind="ExternalInput").ap()
    wext_d = nc.dram_tensor("wext", [F + 1, U4], BF16, kind="ExternalInput").ap()
    u2_d = nc.dram_tensor("u2", [128, 2 * U4], BF16, kind="ExternalInput").ap()
    ud2_d = nc.dram_tensor("ud2", [128, 2 * U4], BF16, kind="ExternalInput").ap()
    bdec_d = nc.dram_tensor("bdec", [1, U4], BF16, kind="ExternalInput").ap()
    wdd_d = nc.dram_tensor("wdd", [128, 2 * F], BF16, kind="ExternalInput").ap()
    bdrow_d = nc.dram_tensor("bdrow", [1, F], BF16, kind="ExternalInput").ap()
    ident_d = nc.dram_tensor("ident", [128, 128], F32, kind="ExternalInput").ap()
    yout = nc.dram_tensor("yout", [B, out_steps, F], F32, kind="ExternalOutput").ap()

    xin_f = xin.rearrange("b t f -> b (t f)")
    yout_f = yout.rearrange("b s f -> b (s f)")

    n_steps = T + out_steps - 1          # total lstm steps (h(T-1+j) -> pred_j)

    with tile.TileContext(nc) as tc:
        import contextlib

        with contextlib.ExitStack() as ctx:
            wpool = ctx.enter_context(tc.tile_pool(name="wpool", bufs=1))
            dpool = ctx.enter_context(tc.tile_pool(name="dpool", bufs=8))
            xspool = ctx.enter_context(tc.tile_pool(name="xspool", bufs=WIN + 2))
            gpool = ctx.enter_context(tc.tile_pool(name="gpool", bufs=3))
            hpool = ctx.enter_context(tc.tile_pool(name="hpool", bufs=3))
            pspool = ctx.enter_context(tc.tile_pool(name="pspool", bufs=2))
            zpool = ctx.enter_context(tc.tile_pool(name="zpool", bufs=3, space="PSUM"))
            upool = ctx.enter_context(tc.tile_pool(name="upool", bufs=2, space="PSUM"))

            # ---- constants / weights ----
            # DMA order: step-0's needs first (ident for transposes, wext for
            # the x matmuls), bulky recurrent weights after
            ident = wpool.tile([128, 128], F32)
            nc.sync.dma_start(ident[:], ident_d[:])
            identb = wpool.tile([128, 128], BF16)
            nc.gpsimd.tensor_copy(identb[:], ident[:])
            wext = wpool.tile([F + 1, U4], BF16)
            nc.sync.dma_start(wext[:], wext_d[:])
            u2 = wpool.tile([128, 2 * U4], BF16)
            nc.sync.dma_start(u2[:], u2_d[:])
            ud2 = wpool.tile([128, 2 * U4], BF16)
            nc.sync.dma_start(ud2[:], ud2_d[:])
            bdec = wpool.tile([1, U4], BF16)
            nc.sync.dma_start(bdec[:], bdec_d[:])
            wdd = wpool.tile([128, 2 * F], BF16)
            nc.sync.dma_start(wdd[:], wdd_d[:])
            bdrow = wpool.tile([1, F], BF16)
            nc.sync.dma_start(bdrow[:], bdrow_d[:])
            ones = wpool.tile([1, B], BF16)
            nc.gpsimd.memset(ones[:], 1.0)

            dts = {}      # pair idx -> list of 4 dt tiles [128,128]
            xss = {}      # step -> xs tile [65, B] bf16 (x^T + ones row)

            def load_pair(p):
                tiles = []
                for bc in range(NB):
                    dt = dpool.tile([128, 128], F32, tag="dt", name=f"dt{p}_{bc}")
                    nc.sync.dma_start(
                        dt[:], xin_f[128 * bc : 128 * (bc + 1), 128 * p : 128 * (p + 1)]
                    )
                    db = dpool.tile([128, 128], BF16, tag="db", name=f"db{p}_{bc}")
                    nc.gpsimd.tensor_copy(db[:], dt[:])
                    tiles.append(db)
                dts[p] = tiles

            def stage_step(s):
                """Transpose x_s into xs[s] = [F+1, B] bf16 (ones row at F)."""
                p, half = s // 2, s % 2
                st = upool.tile([F, B], BF16, tag="u", name=f"st{s}")
                for bc in range(NB):
                    nc.tensor.transpose(
                        st[:, 128 * bc : 128 * (bc + 1)],
                        dts[p][bc][:, F * half : F * (half + 1)],
                        identb[:],
                    )
                xs = xspool.tile([F + 1, B], BF16, tag="xs", name=f"xs{s}")
                nc.gpsimd.memset(xs[F : F + 1, :], 1.0)
                nc.vector.tensor_copy(xs[0:F, :], st[:])
                xss[s] = xs
                if half == 1:
                    del dts[p]

            # gate order on PE / Act: f, i, g, o
            GATES = (G_F, G_I, G_G, G_O)
            zt = {}       # (step, gate) -> z tile [128, 2B] psum

            def mcol(q, ch):
                return 256 * q + 128 * ch

            def emit_zstart(t, q):
                """Allocate z tile for (t, q) and emit its bias/x matmul(s)."""
                zq = zpool.tile([128, B2], F32, tag="z", name=f"z{t}_{q}")
                zt[(t, q)] = zq
                last = t == 0       # step 0 has no recurrent term
                if t < T:
                    for ch in (0, 1):
                        nc.tensor.matmul(
                            zq[:, B * ch : B * (ch + 1)],
                            wext[:, mcol(q, ch) : mcol(q, ch) + 128],
                            xss[t][:],
                            start=True,
                            stop=last,
                        )
                else:
                    for ch in (0, 1):
                        nc.tensor.matmul(
                            zq[:, B * ch : B * (ch + 1)],
                            bdec[:, mcol(q, ch) : mcol(q, ch) + 128],
                            ones[:],
                            start=True,
                            stop=False,
                        )
                return zq

            def emit_kmms(t, q, h_prev):
                """Recurrent matmuls for gate q of step t.  For the f gate the
                ch0 output column is finished first so sig(f0) can start before
                h1 of the previous step is even ready."""
                uw = u2 if t < T else ud2
                zq = zt[(t, q)]
                for k, ch in ((0, 0), (0, 1), (1, 0), (1, 1)):
                    nc.tensor.matmul(
                        zq[:, B * ch : B * (ch + 1)],
                        uw[:, U4 * k + mcol(q, ch) : U4 * k + mcol(q, ch) + 128],
                        h_prev[:, B * k : B * (k + 1)],
                        start=False,
                        stop=(k == 1),
                    )

            def emit_act_dve(t, c_prev):
                """Gate activations + cell update for step t. Returns (h, c)."""
                si = gpool.tile([128, B2], BF16, tag="si", name=f"si{t}")
                tg = gpool.tile([128, B2], BF16, tag="tg", name=f"tg{t}")
                so = gpool.tile([128, B2], BF16, tag="so", name=f"so{t}")
                th = gpool.tile([128, B2], BF16, tag="th", name=f"th{t}")
                c_t = gpool.tile([128, B2], FP16, tag="c", name=f"c{t}")
                h_t = hpool.tile([128, B2], BF16, tag="h", name=f"h{t}")
                if c_prev is not None:
                    sf = gpool.tile([128, B2], BF16, tag="sf", name=f"sf{t}")
                    t1 = gpool.tile([128, B2], BF16, tag="t1", name=f"t1_{t}")
                    fc = gpool.tile([128, B2], FP16, tag="fc", name=f"fc{t}")

                s0 = slice(0, B)
                s1 = slice(B, B2)
                if c_prev is not None:
                    nc.scalar.activation(sf[:], zt.pop((t, G_F))[:], AF.Sigmoid)
                else:
                    zt.pop((t, G_F))   # f gate multiplies c=0: skip
                nc.scalar.activation(si[:], zt.pop((t, G_I))[:], AF.Sigmoid)
                nc.scalar.activation(tg[:], zt.pop((t, G_G))[:], AF.Tanh)
                nc.scalar.activation(so[:], zt.pop((t, G_O))[:], AF.Sigmoid)
                if c_prev is not None:
                    nc.vector.tensor_mul(fc[:, s0], sf[:, s0], c_prev[:, s0])
                    nc.vector.tensor_mul(fc[:, s1], sf[:, s1], c_prev[:, s1])
                    nc.vector.tensor_mul(t1[:, s0], si[:, s0], tg[:, s0])
                    nc.gpsimd.tensor_mul(t1[:, s1], si[:, s1], tg[:, s1])
                    nc.vector.tensor_add(c_t[:, s0], fc[:, s0], t1[:, s0])
                    nc.vector.tensor_add(c_t[:, s1], fc[:, s1], t1[:, s1])
                else:
                    nc.vector.tensor_mul(c_t[:, s0], si[:, s0], tg[:, s0])
                    nc.vector.tensor_mul(c_t[:, s1], si[:, s1], tg[:, s1])
                nc.scalar.activation(th[:, s0], c_t[:, s0], AF.Tanh)
                nc.vector.tensor_mul(h_t[:, s0], so[:, s0], th[:, s0])
                nc.scalar.activation(th[:, s1], c_t[:, s1], AF.Tanh)
                nc.vector.tensor_mul(h_t[:, s1], so[:, s1], th[:, s1])
                return h_t, c_t

            preds = {}    # j -> osb tile [128, NB*F] f32, [batch, feat] layout

            def emit_pred(j, h_t):
                """pred_j = h@Wd + bd computed DIRECTLY in [batch, feat] layout
                by using the h slices as the matmul stationary operand: output
                free dim is just F=64, so each matmul costs ~27ns and no
                transposes are needed downstream."""
                po = upool.tile([128, NB * F], F32, tag="u", name=f"po{j}")
                for bc in range(NB):
                    sl = slice(F * bc, F * (bc + 1))
                    bsl = slice(128 * bc, 128 * (bc + 1))
                    nc.tensor.matmul(
                        po[:, sl], ones[0:1, bsl], bdrow[:], start=True, stop=False
                    )
                    nc.tensor.matmul(
                        po[:, sl], h_t[:, bsl], wdd[:, 0:F], start=False, stop=False
                    )
                    nc.tensor.matmul(
                        po[:, sl],
                        h_t[:, B + 128 * bc : B + 128 * (bc + 1)],
                        wdd[:, F : 2 * F],
                        start=False,
                        stop=True,
                    )
                osb = pspool.tile([128, NB * F], F32, tag="ob", name=f"ob{j}")
                nc.vector.tensor_copy(osb[:], po[:])
                preds[j] = osb

            def emit_out(j):
                """DMA pred_j (already [batch, feat]) to the output."""
                osb = preds.pop(j)
                for bc in range(NB):
                    nc.sync.dma_start(
                        yout_f[128 * bc : 128 * (bc + 1), F * j : F * (j + 1)],
                        osb[:, F * bc : F * bc + F],
                    )

            # ================= prologue =================
            for p in range((WIN + 1) // 2 + 1):
                load_pair(p)
            for s in range(WIN):
                stage_step(s)
            for q in GATES:
                emit_zstart(0, q)

            # ================= main loop =================
            h_t = c_t = None
            hs = {}
            for t in range(n_steps):
                h_prev, c_prev = h_t, c_t
                if t > 0:
                    # k-chunk matmuls for f, i, g; o's z-start (slot frees after
                    # sig(f) of this step) then o's k-matmuls.
                    emit_kmms(t, G_F, h_prev)
                    emit_kmms(t, G_I, h_prev)
                    emit_kmms(t, G_G, h_prev)
                    emit_zstart(t, G_O)
                    emit_kmms(t, G_O, h_prev)
                # 2-step-lagged pred: matmuls run right after the k-wave, so
                # the pred PSUM->SBUF copy is ready late (DVE idle window)
                if t - 1 >= T:
                    emit_pred(t - 1 - T, hs.pop(t - 2))


                # PE filler work for the recurrence tail: input staging, next
                # step's bias/x matmuls, then the decode output path (after
                # the x matmuls so its PSUM->SBUF copy lands in the DVE's
                # idle window instead of delaying the cell-update chain).
                s = t + WIN
                if s < T:
                    if s % 2 == 0 and s // 2 + 1 < (T + 1) // 2:
                        load_pair(s // 2 + 1)
                    stage_step(s)
                if t > T and t - T >= 2:
                    emit_out(t - T - 2)
                if t + 1 < n_steps:
                    emit_zstart(t + 1, G_F)
                    emit_zstart(t + 1, G_I)
                    emit_zstart(t + 1, G_G)

                h_t, c_t = emit_act_dve(t, c_prev)
                if t >= T - 1:
                    hs[t] = h_t

            # ================= epilogue =================
            for j in (out_steps - 2, out_steps - 1):
                emit_pred(j, hs.pop(j + T - 1))
            for j in range(out_steps - 4, out_steps):
                if j in preds:
                    emit_out(j)

    nc.compile()
    return nc


_CACHE = {}


def _get_program(key):
    if key not in _CACHE:
        _CACHE[key] = build_program(*key)
    return _CACHE[key]


def _host_prep(W, Uk, b, Wd, bd):
    bf = mybir.dt.np(BF16)
    W64 = W.astype(np.float64)
    Ud = (Uk.astype(np.float64) + Wd.astype(np.float64) @ W64).astype(np.float32)
    bdec = (b.astype(np.float64) + bd.astype(np.float64) @ W64).astype(np.float32)
    wext = np.concatenate([W, b.reshape(1, -1)], axis=0)          # [65, 1024]
    u2 = np.concatenate([Uk[0:128], Uk[128:256]], axis=1)         # [128, 2048]
    ud2 = np.concatenate([Ud[0:128], Ud[128:256]], axis=1)
    wdd = np.concatenate([Wd[0:128], Wd[128:256]], axis=1)        # [128, 128]
    ident = np.eye(128, dtype=np.float32)
    return {
        "wext": wext.astype(bf),
        "u2": u2.astype(bf),
        "ud2": ud2.astype(bf),
        "bdec": np.ascontiguousarray(bdec.reshape(1, -1)).astype(bf),
        "wdd": wdd.astype(bf),
        "bdrow": np.ascontiguousarray(bd.reshape(1, -1)).astype(bf),
        "ident": ident,
    }


def kernel(inputs, W, U, b, Wd, bd, out_steps):
    inputs = np.asarray(inputs, dtype=np.float32)
    W = np.asarray(W, dtype=np.float32)
    U_ = np.asarray(U, dtype=np.float32)
    b_ = np.asarray(b, dtype=np.float32)
    Wd = np.asarray(Wd, dtype=np.float32)
    bd = np.asarray(bd, dtype=np.float32)
    out_steps = int(out_steps)

    B_full, T, _ = inputs.shape
    assert B_full % N_CORES == 0
    Bc = B_full // N_CORES

    nc = _get_program((Bc, T, out_steps))
    shared = _host_prep(W, U_, b_, Wd, bd)
    in_maps = [
        {"xin": np.ascontiguousarray(inputs[i * Bc : (i + 1) * Bc]), **shared}
        for i in range(N_CORES)
    ]
    res = bass_utils.run_bass_kernel_spmd(nc, in_maps, core_ids=list(range(N_CORES)))
    out = np.concatenate([res.results[i]["yout"] for i in range(N_CORES)], axis=0)
    return out
